# revision 1
# baseline (speedup 1.0000x reference)
"""ErnieLayout self-attention on 8 Trainium2 NeuronCores (Bass/Tile).

Problem shapes (hardcoded): B=4, S=1024, H=768, NH=12, HD=64.
Sharding: core c -> (batch b = c//2, head-half hh = c%2, i.e. 6 heads).
Each core computes attention for its 6 heads of one batch element and
writes the [S, 384] column slice of that batch's output.

Per-core algorithm (mixed precision, scores kept transposed):
  setup:  X and W cast to fp16 (DVE), transposed on the PE (fp16 path);
          Q^T = (Wq_s @ X^T + bq)/8, K^T = Wk_s @ X^T + bk   (fp16 matmuls,
          fp32 PSUM accumulate, fp16 output tiles)
          V = X @ Wv_s^T (+ bv via DVE broadcast add), stored fp16 with a
          ones column appended (col 64 -> softmax denominator for free)
  per (head, ktile, q-chunk):
          psum[k=128, q=512] = K^T.T @ Q^T               (fp16, 1 cyc/row)
          psum += rel12[q,ktile]^T via matmul(lhsT=rel12_f16, rhs=I_f16)
          pT = exp(psum + maskbias[k]) -> fp16   (ACT per-partition bias;
          masked keys get FLT_MIN so exp underflows to exactly 0, matching
          the reference's FLT_MIN replacement; no row-max needed, scores
          are O(10))
  per head (PV flipped so V is the stationary operand and the fp16 pT
  strips stream at N=512):
          ctx^T[d|1, q-chunk] += V_aug[kt].T @ pT[kt]  over kt
          ctx^T -> SBUF (ACT), back-transposed on the PE (fp32, exact),
          out[q, h*64+d] = ctx[q, d] * (1 / ctx[q, 64])  (DVE recip + ACT
          scale; the finalize of head h is emitted inside head h+1's loop
          so the in-order PE stream never stalls on it)

rel12 = rel_pos + rel_2d_pos is one DVE pass (fp32 in, fp16 out) over
[128, 1024] strips streamed continuously from t=0 (the rel pools are
allocated before the phase-1 pools so their SBUF is disjoint).
Precision: fp16 carries 10 mantissa bits -> final rel err ~1e-3.
"""

import os
import sys

import numpy as np

for _p in ("/opt/trn_rl_repo",):
    if _p not in sys.path and os.path.isdir(_p):
        sys.path.append(_p)

import concourse.bass as bass
import concourse.mybir as mybir
import concourse.tile as tile
from concourse import bacc
from concourse.bass_utils import run_bass_kernel_spmd
from concourse.masks import make_identity

F32 = mybir.dt.float32
F32R = mybir.dt.float32r
F16 = mybir.dt.float16
I32 = mybir.dt.int32
AF = mybir.ActivationFunctionType
NEG = float(np.finfo(np.float32).min)

P = 128
S = 1024
NH = 6        # heads per core
HD = 64
HIN = 768     # model dim (contraction for projections)
HOUT = NH * HD  # 384, per-core projection width
KT = S // P   # 8 key tiles
QT = S // P   # 8 query tiles
VW = HD + 1   # 65: V columns + ones column


def _build_kernel_body(tc, aps):
    import contextlib

    nc = tc.nc
    x_ap = aps["x"]
    mask_ap = aps["mask"]
    rel1_ap = aps["rel1"]
    rel2_ap = aps["rel2"]
    out_ap = aps["out"]

    with contextlib.ExitStack() as ctx:
        const = ctx.enter_context(tc.tile_pool(name="const", bufs=1))

        ident = const.tile([P, P], F16)
        make_identity(nc, ident)
        ident32 = const.tile([P, P], F32)
        nc.vector.tensor_copy(ident32[:], ident[:])


        # long-lived tensors
        qt_pool = ctx.enter_context(tc.tile_pool(name="qT", bufs=3))
        kt_pool = ctx.enter_context(tc.tile_pool(name="kT", bufs=3))
        v_pool = ctx.enter_context(tc.tile_pool(name="v", bufs=8))

        qT = [qt_pool.tile([P, S], F16, tag="qT", name=f"qT{i}") for i in range(3)]
        kT = [kt_pool.tile([P, S], F16, tag="kT", name=f"kT{i}") for i in range(3)]
        v_tiles = [
            v_pool.tile([P, NH, VW], F16, tag="v", name=f"v{i}") for i in range(8)
        ]

        # rel stream pools first: their SBUF is disjoint from phase-1 pools,
        # so rel DMA + DVE adds run from t=0 and deep fp16 buffering keeps
        # the DMA queues fed across head boundaries.
        r1_pool = ctx.enter_context(tc.tile_pool(name="r1", bufs=5))
        r2_pool = ctx.enter_context(tc.tile_pool(name="r2", bufs=5))
        rbf_pool = ctx.enter_context(tc.tile_pool(name="rbf", bufs=32))

        # ---------------- phase 1: load, cast, transpose, project ----------
        with contextlib.ExitStack() as ph1:
            xload = ph1.enter_context(tc.tile_pool(name="xload", bufs=2))
            wload = ph1.enter_context(tc.tile_pool(name="wload", bufs=2))
            x16_pool = ph1.enter_context(tc.tile_pool(name="x16", bufs=8))
            w16_pool = ph1.enter_context(tc.tile_pool(name="w16", bufs=4))
            xt_pool = ph1.enter_context(tc.tile_pool(name="xT", bufs=6))
            wt_pool = ph1.enter_context(tc.tile_pool(name="wT", bufs=18))
            psum1 = ph1.enter_context(tc.tile_pool(name="psum1", bufs=3, space="PSUM"))
            psum1b = ph1.enter_context(
                tc.tile_pool(name="psum1b", bufs=2, space="PSUM")
            )

            # X tiles [128, 768] -> fp16
            x16 = []
            for t in range(8):
                xt_ = xload.tile([P, HIN], F32, tag="x")
                nc.sync.dma_start(xt_[:], x_ap[t * P:(t + 1) * P, :])
                x16_t = x16_pool.tile([P, HIN], F16, tag="x16", name=f"x16_{t}")
                nc.vector.tensor_copy(x16_t[:], xt_[:])
                x16.append(x16_t)

            # mask bias and projection biases: emitted after the X loads so
            # their many-descriptor gather DMAs and DVE ops stay off the
            # startup critical path (only needed from the first exp / proj)
            mask_i = const.tile([P, KT], I32)
            nc.sync.dma_start(mask_i[:], mask_ap.rearrange("(a p) -> p a", p=P))
            maskb = const.tile([P, KT], F32)
            nc.vector.tensor_copy(maskb[:], mask_i[:])
            nc.vector.tensor_scalar_mul(maskb[:], maskb[:], NEG)
            bias_sb = {}
            for wname in ("q", "k"):
                bt = const.tile([P, 3], F32, tag=f"b{wname}")
                nc.sync.dma_start(
                    bt[:], aps[f"b{wname}"].rearrange("(a p) -> p a", p=P)
                )
                if wname == "q":
                    nc.vector.tensor_scalar_mul(bt[:], bt[:], 0.125)
                bias_sb[wname] = bt
            bv_bc = const.tile([P, NH, HD], F32)
            nc.sync.dma_start(
                bv_bc[:],
                aps["bv"].rearrange("(h d) -> h d", d=HD)[None].to_broadcast(
                    (P, NH, HD)
                ),
            )

            # X^T: 6 fp16 tiles [128, 1024] (h-chunk on partitions)
            xT = []
            for hc in range(6):
                pt = psum1.tile([P, S], F16, tag="xtp")  # 1 bank (fp16)
                for t in range(8):
                    nc.tensor.transpose(
                        pt[:, t * P:(t + 1) * P],
                        x16[t][:, hc * P:(hc + 1) * P],
                        ident[:],
                    )
                xt_t = xt_pool.tile([P, S], F16, tag="xT")
                nc.scalar.copy(xt_t[:], pt[:])
                xT.append(xt_t)

            # W^T slices (fp16): wT[(w, hc)] = [128, 384]
            wT = {}
            for wname in ("q", "k", "v"):
                w_ap = aps[f"w{wname}"]
                w16s = []
                for d in range(3):
                    wt_ = wload.tile([P, HIN], F32, tag="wload")
                    nc.sync.dma_start(wt_[:], w_ap[d * P:(d + 1) * P, :])
                    w16_t = w16_pool.tile(
                        [P, HIN], F16, tag="w16", name=f"w16{wname}_{d}"
                    )
                    nc.vector.tensor_copy(w16_t[:], wt_[:])
                    w16s.append(w16_t)
                for hc in range(6):
                    pw = psum1b.tile([P, 512], F16, tag="ps1b", name="pw")[:, :HOUT]
                    for d in range(3):
                        nc.tensor.transpose(
                            pw[:, d * P:(d + 1) * P],
                            w16s[d][:, hc * P:(hc + 1) * P],
                            ident[:],
                        )
                    wt_t = wt_pool.tile([P, HOUT], F16, tag="wT")
                    nc.scalar.copy(wt_t[:], pw[:])
                    wT[(wname, hc)] = wt_t

            # Q^T, K^T projections: fp16 matmuls, fp32 PSUM, fp32r output
            for wname, dest, scale in (("q", qT, 0.125), ("k", kT, 1.0)):
                for d in range(3):
                    for tch in range(2):
                        pp = psum1b.tile([P, 512], F32, tag="projp")
                        for hc in range(6):
                            nc.tensor.matmul(
                                pp[:],
                                wT[(wname, hc)][:, d * P:(d + 1) * P],
                                xT[hc][:, tch * 512:(tch + 1) * 512],
                                start=(hc == 0),
                                stop=(hc == 5),
                            )
                        nc.scalar.activation(
                            dest[d][:, tch * 512:(tch + 1) * 512],
                            pp[:],
                            AF.Identity,
                            bias=bias_sb[wname][:, d:d + 1],
                            scale=scale,
                        )

            # V projection: out [t-tile 128, 384] fp16 + ones column
            for t in range(8):
                pv = psum1b.tile([P, 512], F32, tag="projp", name="pv")[:, :HOUT]
                for hc in range(6):
                    nc.tensor.matmul(
                        pv[:],
                        xT[hc][:, t * P:(t + 1) * P],
                        wT[("v", hc)][:],
                        start=(hc == 0),
                        stop=(hc == 5),
                    )
                nc.vector.memset(v_tiles[t][:], 1.0)
                # copy + bias add (bv broadcast along partitions)
                nc.vector.tensor_add(
                    v_tiles[t][:, :, 0:HD],
                    pv[:].rearrange("p (h d) -> p h d", d=HD),
                    bv_bc[:],
                )

        # ---------------- phase 2: attention per head ----------------
        out_pool = ctx.enter_context(tc.tile_pool(name="outst", bufs=8))
        out_stage = [
            out_pool.tile([P, HOUT], F32, tag="outst", name=f"outst{i}")
            for i in range(8)
        ]
        pt_pool = ctx.enter_context(tc.tile_pool(name="pT", bufs=18))
        fin_pool = ctx.enter_context(tc.tile_pool(name="fin", bufs=4))
        spsum = ctx.enter_context(tc.tile_pool(name="spsum", bufs=4, space="PSUM"))
        vpsum = ctx.enter_context(tc.tile_pool(name="vpsum", bufs=4, space="PSUM"))
        ctt_pool = ctx.enter_context(tc.tile_pool(name="ctt", bufs=2))

        def emit_finalize(h, ctxT_ps):
            """Epilogue for head h: copy ctx^T out of PSUM, back-transpose to
            [q, 65], divide by the denominator. Deferred one head so the
            in-order PE stream never stalls waiting on the ACT copies."""
            ctxT_sb = [None, None]
            for qch in range(2):
                t_ = ctt_pool.tile([VW, 512], F32, tag="ctxT_sb",
                                   name=f"ctxTs{h}_{qch}")
                nc.scalar.copy(t_[:], ctxT_ps[qch][:])
                ctxT_sb[qch] = t_
            ctx_ps = [
                spsum.tile([P, 512], F32, tag="sT", name=f"ctx{h}_{i}")
                for i in range(2)
            ]
            # all PE transposes first, then all DVE reads: avoids the
            # per-slot PE-write/DVE-read same-bank ping-pong serialization
            for qt in range(QT):
                cp = ctx_ps[qt // 4]
                sl = (qt % 4) * VW
                nc.tensor.transpose(
                    cp[:, sl:sl + VW],
                    ctxT_sb[qt // 4][:, (qt % 4) * P:(qt % 4 + 1) * P],
                    ident32[:VW, :VW],
                )
            for qt in range(QT):
                cp = ctx_ps[qt // 4]
                sl = (qt % 4) * VW
                rc = fin_pool.tile([P, 1], F32, tag="recip")
                nc.vector.reciprocal(rc[:], cp[:, sl + HD:sl + HD + 1])
                nc.scalar.activation(
                    out_stage[qt][:, h * HD:(h + 1) * HD],
                    cp[:, sl:sl + HD],
                    AF.Identity,
                    scale=rc[:],
                )

        pending_fin = None
        for h in range(NH):
            # rel12 = rel1 + rel2 -> fp16, eight strip tiles [128, 1024]
            strips = []
            for qq in range(8):
                r1 = r1_pool.tile([P, S], F32, tag="r1")
                nc.sync.dma_start(
                    r1[:],
                    rel1_ap[h].rearrange("(qt p) k -> p qt k", p=P)[:, qq, :],
                )
                r2 = r2_pool.tile([P, S], F32, tag="r2")
                nc.sync.dma_start(
                    r2[:],
                    rel2_ap[h].rearrange("(qt p) k -> p qt k", p=P)[:, qq, :],
                )
                rb = rbf_pool.tile([P, S], F16, tag="rbf", name=f"rbf{h}_{qq}")
                nc.vector.tensor_add(rb[:], r1[:], r2[:])
                strips.append(rb)

            dt, rem = divmod(h, 2)
            d0 = rem * HD
            qTh = qT[dt][d0:d0 + HD, :]
            kTh = kT[dt][d0:d0 + HD, :]

            pT_strips = []
            for kt in range(KT):
                pT_strip = pt_pool.tile([P, S], F16, tag="pT", name=f"pT{h}_{kt}")
                pT_strips.append(pT_strip)
                for qch in range(2):
                    ps = spsum.tile([P, 512], F32, tag="sT")
                    # qk^T (fp32r: full-rate single-pass matmul)
                    nc.tensor.matmul(
                        ps[:],
                        kTh[:, kt * P:(kt + 1) * P],
                        qTh[:, qch * 512:(qch + 1) * 512],
                        start=True,
                        stop=False,
                    )
                    # += rel12^T (transposing adds via fp16 identity rhs)
                    for j in range(4):
                        qt = qch * 4 + j
                        nc.tensor.matmul(
                            ps[:, j * P:(j + 1) * P],
                            strips[qt][:, kt * P:(kt + 1) * P],
                            ident[:],
                            start=False,
                            stop=(j == 3),
                        )
                    # exp(scores + mask bias) -> fp16 probs
                    nc.scalar.activation(
                        pT_strip[:, qch * 512:(qch + 1) * 512],
                        ps[:],
                        AF.Exp,
                        bias=maskb[:, kt:kt + 1],
                        scale=1.0,
                    )
                if kt == 0 and pending_fin is not None:
                    emit_finalize(*pending_fin)
                    pending_fin = None

            # PV flipped: ctx^T[d|1, q] = V_aug.T @ P^T, accumulated over kt.
            # lhsT = V_aug tile (65 cols), rhs = pT strip (N=512 fp16) --
            # 16 big matmuls per head instead of 64 small ones. Row 64 of
            # ctx^T is the softmax denominator (ones column of V_aug).
            ctxT_ps = [
                vpsum.tile([VW, 512], F32, tag="ctxT", name=f"ctxT{h}_{i}")
                for i in range(2)
            ]
            for qch in range(2):
                for kt in range(KT):
                    nc.tensor.matmul(
                        ctxT_ps[qch][:],
                        v_tiles[kt][:, h, :],
                        pT_strips[kt][:, qch * 512:(qch + 1) * 512],
                        start=(kt == 0),
                        stop=(kt == KT - 1),
                    )
            pending_fin = (h, ctxT_ps)

        emit_finalize(*pending_fin)

        for qt in range(QT):
            nc.sync.dma_start(out_ap[qt * P:(qt + 1) * P, :], out_stage[qt][:])



def build_program():
    """Build and compile the per-core Bass program. Returns nc."""
    nc = bacc.Bacc(
        "TRN2",
        target_bir_lowering=False,
        debug=False,
        num_devices=8,
    )
    aps = {
        "x": nc.dram_tensor("x", [S, HIN], F32, kind="ExternalInput").ap(),
        "mask": nc.dram_tensor("mask", [S], I32, kind="ExternalInput").ap(),
        "rel1": nc.dram_tensor("rel1", [NH, S, S], F32, kind="ExternalInput").ap(),
        "rel2": nc.dram_tensor("rel2", [NH, S, S], F32, kind="ExternalInput").ap(),
        "wq": nc.dram_tensor("wq", [HOUT, HIN], F32, kind="ExternalInput").ap(),
        "wk": nc.dram_tensor("wk", [HOUT, HIN], F32, kind="ExternalInput").ap(),
        "wv": nc.dram_tensor("wv", [HOUT, HIN], F32, kind="ExternalInput").ap(),
        "bq": nc.dram_tensor("bq", [HOUT], F32, kind="ExternalInput").ap(),
        "bk": nc.dram_tensor("bk", [HOUT], F32, kind="ExternalInput").ap(),
        "bv": nc.dram_tensor("bv", [HOUT], F32, kind="ExternalInput").ap(),
        "out": nc.dram_tensor("out", [S, HOUT], F32, kind="ExternalOutput").ap(),
    }
    with tile.TileContext(nc) as tc:
        _build_kernel_body(tc, aps)
    nc.compile()
    return nc


def make_in_maps(inputs):
    """Slice full inputs into the 8 per-core input maps."""
    hs = np.ascontiguousarray(np.asarray(inputs["hidden_states"], np.float32))
    am = np.asarray(inputs["attention_mask"]).astype(np.int32)
    rel1 = np.asarray(inputs["rel_pos"], np.float32)
    rel2 = np.asarray(inputs["rel_2d_pos"], np.float32)
    ws = {k: np.asarray(inputs["W" + k[-1]], np.float32) for k in ("wq", "wk", "wv")}
    bs = {k: np.asarray(inputs["b" + k[-1]], np.float32) for k in ("bq", "bk", "bv")}

    in_maps = []
    for c in range(8):
        b, hh = divmod(c, 2)
        hsl = slice(hh * NH, (hh + 1) * NH)
        csl = slice(hh * HOUT, (hh + 1) * HOUT)
        m = {
            "x": np.ascontiguousarray(hs[b]),
            "mask": np.ascontiguousarray(am[b, 0, 0]),
            "rel1": np.ascontiguousarray(rel1[b, hsl]),
            "rel2": np.ascontiguousarray(rel2[b, hsl]),
        }
        for k in ("wq", "wk", "wv"):
            m[k] = np.ascontiguousarray(ws[k][csl])
        for k in ("bq", "bk", "bv"):
            m[k] = np.ascontiguousarray(bs[k][csl])
        in_maps.append(m)
    return in_maps


def gather_output(results):
    out = np.empty((4, S, HIN), np.float32)
    for c in range(8):
        b, hh = divmod(c, 2)
        out[b, :, hh * HOUT:(hh + 1) * HOUT] = results[c]["out"]
    return out


_NC_CACHE = []


def kernel(**inputs):
    if not _NC_CACHE:
        _NC_CACHE.append(build_program())
    nc = _NC_CACHE[0]
    in_maps = make_in_maps(inputs)
    res = run_bass_kernel_spmd(nc, in_maps, list(range(8)))
    return gather_output(res.results)



# revision 4
# speedup vs baseline: 1.1103x; 1.1103x over previous
"""ErnieLayout self-attention on 8 Trainium2 NeuronCores (Bass/Tile). v2

Problem shapes (hardcoded): B=4, S=1024, H=768, NH=12, HD=64.
Sharding: core c -> (batch b = c//2, head-half hh = c%2, i.e. 6 heads).
Each core computes attention for its 6 heads of one batch element and
writes the [S, 384] column slice of that batch's output.

v2 changes vs v1 (which PE-transposed the rel tensors on-chip):
  * rel_pos / rel_2d_pos are uploaded HOST-TRANSPOSED per head ([k, q]
    layout, a pure layout change done while sharding).  The transposed
    strips land contiguously and are added STRAIGHT INTO the scores
    PSUM by the DVE (2 tensor_add RMW ops per [128,512] block), which
    removes all 384 rel transpose matmuls from the PE stream.
  * heads are processed in pairs (2dt, 2dt+1) whose q/k rows live in
    partitions 0-63 / 64-127 of the same qT/kT tile: the two QK score
    matmuls per (kt, qch) are emitted back-to-back and run CONCURRENTLY
    on the PE via row tiling (auto tile_position from base partitions).
  * finalize is restructured per pair with batched work at the pair
    end so the DVE stream is almost exclusively the rel RMW adds (no
    head-of-line blocking of the next pair's work).

Per-core algorithm (mixed precision, scores kept transposed):
  setup:  X and W cast to fp16 (DVE), transposed on the PE (fp16 path);
          Q^T = (Wq_s @ X^T + bq)/8, K^T = Wk_s @ X^T + bk   (fp16
          matmuls, fp32 PSUM accumulate, fp16 output tiles)
          V = X @ Wv_s^T (+ bv via DVE broadcast add), stored fp16 with
          a ones column appended (col 64 -> softmax denominator free)
  per pair (head A rows 0-63, head B rows 64-127 of qT/kT tile dt):
    per (qch, kt):
          psA[k=128, q=512] = K_A^T.T @ Q_A^T   (concurrent row tiles)
          psB[k=128, q=512] = K_B^T.T @ Q_B^T
          ps += rel1T block + rel2T block        (DVE RMW, fp32)
          pT = exp(ps + maskbias[k]) -> fp16     (ACT per-partition
          bias; masked keys underflow to exactly 0)
    per (head, qch):  ctx^T[d|1, q] += V_aug[kt].T @ pT[kt]  over kt
  fin (deferred one pair): ctx^T -> SBUF (ACT), back-transposed on the
          PE (fp32), out[q, h*64+d] = ctx[q, d] * (1 / ctx[q, 64])
"""

import os
import sys

import numpy as np

for _p in ("/opt/trn_rl_repo",):
    if _p not in sys.path and os.path.isdir(_p):
        sys.path.append(_p)

import concourse.bass as bass
import concourse.mybir as mybir
import concourse.tile as tile
from concourse import bacc
from concourse.bass_utils import run_bass_kernel_spmd
from concourse.masks import make_identity

F32 = mybir.dt.float32
F16 = mybir.dt.float16
I32 = mybir.dt.int32
AF = mybir.ActivationFunctionType
NEG = float(np.finfo(np.float32).min)

P = 128
S = 1024
NH = 6        # heads per core
HD = 64
HIN = 768     # model dim (contraction for projections)
HOUT = NH * HD  # 384, per-core projection width
KT = S // P   # 8 key tiles
QT = S // P   # 8 query tiles
VW = HD + 1   # 65: V columns + ones column
NPAIR = NH // 2


def _build_kernel_body(tc, aps):
    import contextlib

    nc = tc.nc
    x_ap = aps["x"]
    mask_ap = aps["mask"]
    rel1_ap = aps["rel1"]  # [NH, S(k), S(q)] -- host-transposed
    rel2_ap = aps["rel2"]
    out_ap = aps["out"]

    with contextlib.ExitStack() as ctx:
        const = ctx.enter_context(tc.tile_pool(name="const", bufs=1))

        ident = const.tile([P, P], F16)
        make_identity(nc, ident)
        ident32 = const.tile([P, P], F32)
        nc.vector.tensor_copy(ident32[:], ident[:])

        # long-lived tensors
        qt_pool = ctx.enter_context(tc.tile_pool(name="qT", bufs=3))
        kt_pool = ctx.enter_context(tc.tile_pool(name="kT", bufs=3))
        v_pool = ctx.enter_context(tc.tile_pool(name="v", bufs=8))

        qT = [qt_pool.tile([P, S], F16, tag="qT", name=f"qT{i}") for i in range(3)]
        kT = [kt_pool.tile([P, S], F16, tag="kT", name=f"kT{i}") for i in range(3)]
        v_tiles = [
            v_pool.tile([P, NH, VW], F16, tag="v", name=f"v{i}") for i in range(8)
        ]

        # rel strip pool allocated before the phase-1 pools so its SBUF is
        # disjoint: strip DMAs queue behind the x/W loads and stream
        # continuously for the whole kernel.
        r_pool = ctx.enter_context(tc.tile_pool(name="rel", bufs=20))

        # ---------------- phase 1: load, cast, transpose, project ----------
        with contextlib.ExitStack() as ph1:
            xload = ph1.enter_context(tc.tile_pool(name="xload", bufs=2))
            wload = ph1.enter_context(tc.tile_pool(name="wload", bufs=2))
            x16_pool = ph1.enter_context(tc.tile_pool(name="x16", bufs=8))
            w16_pool = ph1.enter_context(tc.tile_pool(name="w16", bufs=4))
            xt_pool = ph1.enter_context(tc.tile_pool(name="xT", bufs=6))
            wt_pool = ph1.enter_context(tc.tile_pool(name="wT", bufs=18))
            psum1 = ph1.enter_context(tc.tile_pool(name="psum1", bufs=3, space="PSUM"))
            psum1b = ph1.enter_context(
                tc.tile_pool(name="psum1b", bufs=2, space="PSUM")
            )

            # X tiles [128, 768] -> fp16
            x16 = []
            for t in range(8):
                xt_ = xload.tile([P, HIN], F32, tag="x")
                nc.sync.dma_start(xt_[:], x_ap[t * P:(t + 1) * P, :])
                x16_t = x16_pool.tile([P, HIN], F16, tag="x16", name=f"x16_{t}")
                nc.vector.tensor_copy(x16_t[:], xt_[:])
                x16.append(x16_t)

            # mask bias and projection biases after the X loads (off the
            # startup critical path)
            mask_i = const.tile([P, KT], I32)
            nc.sync.dma_start(mask_i[:], mask_ap.rearrange("(a p) -> p a", p=P))
            maskb = const.tile([P, KT], F32)
            nc.vector.tensor_copy(maskb[:], mask_i[:])
            nc.vector.tensor_scalar_mul(maskb[:], maskb[:], NEG)
            bias_sb = {}
            for wname in ("q", "k"):
                bt = const.tile([P, 3], F32, tag=f"b{wname}")
                nc.sync.dma_start(
                    bt[:], aps[f"b{wname}"].rearrange("(a p) -> p a", p=P)
                )
                if wname == "q":
                    nc.vector.tensor_scalar_mul(bt[:], bt[:], 0.125)
                bias_sb[wname] = bt
            bv_bc = const.tile([P, NH, HD], F32)
            nc.sync.dma_start(
                bv_bc[:],
                aps["bv"].rearrange("(h d) -> h d", d=HD)[None].to_broadcast(
                    (P, NH, HD)
                ),
            )

            # X^T: 6 fp16 tiles [128, 1024] (h-chunk on partitions)
            xT = []
            for hc in range(6):
                pt = psum1.tile([P, S], F16, tag="xtp")  # 1 bank (fp16)
                for t in range(8):
                    nc.tensor.transpose(
                        pt[:, t * P:(t + 1) * P],
                        x16[t][:, hc * P:(hc + 1) * P],
                        ident[:],
                    )
                xt_t = xt_pool.tile([P, S], F16, tag="xT")
                nc.scalar.copy(xt_t[:], pt[:])
                xT.append(xt_t)

            # W^T slices (fp16): wT[(w, hc)] = [128, 384]
            wT = {}
            for wname in ("q", "k", "v"):
                w_ap = aps[f"w{wname}"]
                w16s = []
                for d in range(3):
                    wt_ = wload.tile([P, HIN], F32, tag="wload")
                    nc.sync.dma_start(wt_[:], w_ap[d * P:(d + 1) * P, :])
                    w16_t = w16_pool.tile(
                        [P, HIN], F16, tag="w16", name=f"w16{wname}_{d}"
                    )
                    nc.vector.tensor_copy(w16_t[:], wt_[:])
                    w16s.append(w16_t)
                for hc in range(6):
                    pw = psum1b.tile([P, 512], F16, tag="ps1b", name="pw")[:, :HOUT]
                    for d in range(3):
                        nc.tensor.transpose(
                            pw[:, d * P:(d + 1) * P],
                            w16s[d][:, hc * P:(hc + 1) * P],
                            ident[:],
                        )
                    wt_t = wt_pool.tile([P, HOUT], F16, tag="wT")
                    nc.scalar.copy(wt_t[:], pw[:])
                    wT[(wname, hc)] = wt_t

            # Q^T, K^T projections: fp16 matmuls, fp32 PSUM, fp16 output
            for wname, dest, scale in (("q", qT, 0.125), ("k", kT, 1.0)):
                for d in range(3):
                    for tch in range(2):
                        pp = psum1b.tile([P, 512], F32, tag="projp")
                        for hc in range(6):
                            nc.tensor.matmul(
                                pp[:],
                                wT[(wname, hc)][:, d * P:(d + 1) * P],
                                xT[hc][:, tch * 512:(tch + 1) * 512],
                                start=(hc == 0),
                                stop=(hc == 5),
                            )
                        nc.scalar.activation(
                            dest[d][:, tch * 512:(tch + 1) * 512],
                            pp[:],
                            AF.Identity,
                            bias=bias_sb[wname][:, d:d + 1],
                            scale=scale,
                        )

            # V projection: out [t-tile 128, 384] fp16 + ones column
            for t in range(8):
                pv = psum1b.tile([P, 512], F32, tag="projp", name="pv")[:, :HOUT]
                for hc in range(6):
                    nc.tensor.matmul(
                        pv[:],
                        xT[hc][:, t * P:(t + 1) * P],
                        wT[("v", hc)][:],
                        start=(hc == 0),
                        stop=(hc == 5),
                    )
                nc.vector.memset(v_tiles[t][:], 1.0)
                nc.vector.tensor_add(
                    v_tiles[t][:, :, 0:HD],
                    pv[:].rearrange("p (h d) -> p h d", d=HD),
                    bv_bc[:],
                )

        # ---------------- phase 2: attention per head pair -----------------
        out_pool = ctx.enter_context(tc.tile_pool(name="outst", bufs=8))
        out_stage = [
            out_pool.tile([P, HOUT], F32, tag="outst", name=f"outst{i}")
            for i in range(8)
        ]
        pt_pool = ctx.enter_context(tc.tile_pool(name="pT", bufs=20))
        fin_pool = ctx.enter_context(tc.tile_pool(name="fin", bufs=4))
        ctt_pool = ctx.enter_context(tc.tile_pool(name="ctt", bufs=4))
        spsum = ctx.enter_context(tc.tile_pool(name="spsum", bufs=4, space="PSUM"))
        vpsum = ctx.enter_context(tc.tile_pool(name="vpsum", bufs=4, space="PSUM"))

        def emit_fin_copy(fin):
            """ACT-copy the previous pair's ctx^T accumulators out of PSUM
            (releases the vpsum banks for this pair's PV groups)."""
            dt, ctxT_ps = fin
            ctxT_sb = {}
            for h2 in range(2):
                for qch in range(2):
                    t_ = ctt_pool.tile(
                        [VW, 512], F32, tag="ctxT_sb", name=f"ctT{dt}_{h2}_{qch}"
                    )
                    nc.scalar.copy(t_[:], ctxT_ps[(h2, qch)][:])
                    ctxT_sb[(h2, qch)] = t_
            return ctxT_sb

        def emit_fin_rest(fin, ctxT_sb, emit_out_dma):
            """Back-transpose ctx^T per head, divide by the denominator,
            write out_stage (and the output DMAs for the last pair)."""
            dt, _ = fin
            for h2 in range(2):
                h = 2 * dt + h2
                ctx_ps = [
                    spsum.tile([P, 512], F32, tag="sT", name=f"ctx{h}_{i}")
                    for i in range(2)
                ]
                for qt in range(QT):
                    cp = ctx_ps[qt // 4]
                    sl = (qt % 4) * VW
                    nc.tensor.transpose(
                        cp[:, sl:sl + VW],
                        ctxT_sb[(h2, qt // 4)][:, (qt % 4) * P:(qt % 4 + 1) * P],
                        ident32[:VW, :VW],
                    )
                for qt in range(QT):
                    cp = ctx_ps[qt // 4]
                    sl = (qt % 4) * VW
                    rc = fin_pool.tile([P, 1], F32, tag="recip")
                    nc.vector.reciprocal(rc[:], cp[:, sl + HD:sl + HD + 1])
                    nc.scalar.activation(
                        out_stage[qt][:, h * HD:(h + 1) * HD],
                        cp[:, sl:sl + HD],
                        AF.Identity,
                        scale=rc[:],
                    )
                    if emit_out_dma and h2 == 1:
                        nc.sync.dma_start(
                            out_ap[qt * P:(qt + 1) * P, :], out_stage[qt][:]
                        )

        pending_fin = None
        for dt in range(NPAIR):
            # rel strips for both heads of the pair: [k=128, q=1024] fp32,
            # kt-major with heads interleaved to match consumption order.
            r1 = [[None] * KT for _ in range(2)]
            r2 = [[None] * KT for _ in range(2)]
            for kt in range(KT):
                for h2 in range(2):
                    h = 2 * dt + h2
                    t1 = r_pool.tile([P, S], F32, tag="rel", name=f"r1_{h}_{kt}")
                    nc.sync.dma_start(t1[:], rel1_ap[h][kt * P:(kt + 1) * P, :])
                    r1[h2][kt] = t1
                    t2 = r_pool.tile([P, S], F32, tag="rel", name=f"r2_{h}_{kt}")
                    nc.sync.dma_start(t2[:], rel2_ap[h][kt * P:(kt + 1) * P, :])
                    r2[h2][kt] = t2

            if pending_fin is not None:
                fin_sb = emit_fin_copy(pending_fin)

            # score strips: pT[h2][kt] [128(k), 1024(q)] fp16
            pT_strips = [
                [
                    pt_pool.tile([P, S], F16, tag="pT", name=f"pT{dt}_{h2}_{kt}")
                    for kt in range(KT)
                ]
                for h2 in range(2)
            ]
            # kt-outer: each strip pair is fully consumed (both q-chunks)
            # within its kt block, so the 20-deep rel pool never blocks the
            # DMA queue on far-future readers.
            for kt in range(KT):
                ps4 = {}
                for qch in range(2):
                    qsl = slice(qch * 512, (qch + 1) * 512)
                    for h2 in range(2):
                        d0 = h2 * HD
                        ps = spsum.tile([P, 512], F32, tag="sT")
                        # back-to-back K=64 matmuls at base partitions 0/64
                        # get distinct row-group tile_positions -> they run
                        # concurrently on the PE array
                        nc.tensor.matmul(
                            ps[:],
                            kT[dt][d0:d0 + HD, kt * P:(kt + 1) * P],
                            qT[dt][d0:d0 + HD, qsl],
                            start=True,
                            stop=True,
                        )
                        ps4[(qch, h2)] = ps
                for qch in range(2):
                    qsl = slice(qch * 512, (qch + 1) * 512)
                    for h2 in range(2):
                        ps = ps4[(qch, h2)]
                        nc.vector.tensor_add(ps[:], ps[:], r1[h2][kt][:, qsl])
                        nc.vector.tensor_add(ps[:], ps[:], r2[h2][kt][:, qsl])
                        nc.scalar.activation(
                            pT_strips[h2][kt][:, qsl],
                            ps[:],
                            AF.Exp,
                            bias=maskb[:, kt:kt + 1],
                            scale=1.0,
                        )
            # PV (both heads x both q-chunks), V stationary, fp16
            ctxT_ps = {}
            for qch in range(2):
                qsl = slice(qch * 512, (qch + 1) * 512)
                for h2 in range(2):
                    h = 2 * dt + h2
                    cp = vpsum.tile([VW, 512], F32, tag="ctxT",
                                    name=f"ctxT{dt}_{h2}_{qch}")
                    for kt in range(KT):
                        nc.tensor.matmul(
                            cp[:],
                            v_tiles[kt][:, h, :],
                            pT_strips[h2][kt][:, qsl],
                            start=(kt == 0),
                            stop=(kt == KT - 1),
                        )
                    ctxT_ps[(h2, qch)] = cp

            if pending_fin is not None:
                emit_fin_rest(pending_fin, fin_sb, emit_out_dma=False)
            pending_fin = (dt, ctxT_ps)

        fin_sb = emit_fin_copy(pending_fin)
        emit_fin_rest(pending_fin, fin_sb, emit_out_dma=True)


def build_program():
    """Build and compile the per-core Bass program. Returns nc."""
    nc = bacc.Bacc(
        "TRN2",
        target_bir_lowering=False,
        debug=False,
        num_devices=8,
    )
    aps = {
        "x": nc.dram_tensor("x", [S, HIN], F32, kind="ExternalInput").ap(),
        "mask": nc.dram_tensor("mask", [S], I32, kind="ExternalInput").ap(),
        "rel1": nc.dram_tensor("rel1", [NH, S, S], F32, kind="ExternalInput").ap(),
        "rel2": nc.dram_tensor("rel2", [NH, S, S], F32, kind="ExternalInput").ap(),
        "wq": nc.dram_tensor("wq", [HOUT, HIN], F32, kind="ExternalInput").ap(),
        "wk": nc.dram_tensor("wk", [HOUT, HIN], F32, kind="ExternalInput").ap(),
        "wv": nc.dram_tensor("wv", [HOUT, HIN], F32, kind="ExternalInput").ap(),
        "bq": nc.dram_tensor("bq", [HOUT], F32, kind="ExternalInput").ap(),
        "bk": nc.dram_tensor("bk", [HOUT], F32, kind="ExternalInput").ap(),
        "bv": nc.dram_tensor("bv", [HOUT], F32, kind="ExternalInput").ap(),
        "out": nc.dram_tensor("out", [S, HOUT], F32, kind="ExternalOutput").ap(),
    }
    with tile.TileContext(nc) as tc:
        _build_kernel_body(tc, aps)
    nc.compile()
    return nc


def make_in_maps(inputs):
    """Slice full inputs into the 8 per-core input maps.

    rel_pos / rel_2d_pos are uploaded transposed per head ([k, q] layout)
    so their strips add directly into the transposed score tiles."""
    hs = np.ascontiguousarray(np.asarray(inputs["hidden_states"], np.float32))
    am = np.asarray(inputs["attention_mask"]).astype(np.int32)
    rel1 = np.asarray(inputs["rel_pos"], np.float32)
    rel2 = np.asarray(inputs["rel_2d_pos"], np.float32)
    ws = {k: np.asarray(inputs["W" + k[-1]], np.float32) for k in ("wq", "wk", "wv")}
    bs = {k: np.asarray(inputs["b" + k[-1]], np.float32) for k in ("bq", "bk", "bv")}

    rel1T = np.ascontiguousarray(rel1.transpose(0, 1, 3, 2))
    rel2T = np.ascontiguousarray(rel2.transpose(0, 1, 3, 2))

    in_maps = []
    for c in range(8):
        b, hh = divmod(c, 2)
        hsl = slice(hh * NH, (hh + 1) * NH)
        csl = slice(hh * HOUT, (hh + 1) * HOUT)
        m = {
            "x": np.ascontiguousarray(hs[b]),
            "mask": np.ascontiguousarray(am[b, 0, 0]),
            "rel1": np.ascontiguousarray(rel1T[b, hsl]),
            "rel2": np.ascontiguousarray(rel2T[b, hsl]),
        }
        for k in ("wq", "wk", "wv"):
            m[k] = np.ascontiguousarray(ws[k][csl])
        for k in ("bq", "bk", "bv"):
            m[k] = np.ascontiguousarray(bs[k][csl])
        in_maps.append(m)
    return in_maps


def gather_output(results):
    out = np.empty((4, S, HIN), np.float32)
    for c in range(8):
        b, hh = divmod(c, 2)
        out[b, :, hh * HOUT:(hh + 1) * HOUT] = results[c]["out"]
    return out


_NC_CACHE = []


def kernel(**inputs):
    if not _NC_CACHE:
        _NC_CACHE.append(build_program())
    nc = _NC_CACHE[0]
    in_maps = make_in_maps(inputs)
    res = run_bass_kernel_spmd(nc, in_maps, list(range(8)))
    return gather_output(res.results)


# revision 8
# speedup vs baseline: 1.1435x; 1.0300x over previous
"""ErnieLayout self-attention on 8 Trainium2 NeuronCores (Bass/Tile). v3

Problem shapes (hardcoded): B=4, S=1024, H=768, NH=12, HD=64.
Sharding: core c -> (batch b = c//2, head-half hh = c%2, i.e. 6 heads).
Each core computes attention for its 6 heads of one batch element and
writes the [S, 384] column slice of that batch's output.

The kernel is HBM-bound: rel_pos + rel_2d_pos are 50.3 MB per core of
the ~58 MB total I/O, so the design keeps the DMA queues saturated and
sizes every engine's work under the ~160 us DMA floor (robust even when
the PE is power-throttled to 1.2 GHz, which traces show happens for most
of the kernel).

Key structure:
  * rel_pos / rel_2d_pos are uploaded HOST-TRANSPOSED per head ([k, q]
    layout, a pure layout change done while sharding).  Strips land
    contiguously; GPSIMD pre-sums rel1+rel2 in place (idle engine), and
    the DVE adds the sum straight into the transposed score PSUM with
    one RMW per [128,512] block.  No PE transposes of rel at all.
  * heads are processed in pairs (2dt, 2dt+1) whose q/k rows live in
    partitions 0-63 / 64-127 of qT/kT tile dt: the two QK score matmuls
    per (kt, qch) are emitted back-to-back and run CONCURRENTLY on the
    PE via row tiling (auto tile_position from base partitions).
  * only the X/W transposes and the d=0 Q/K projections run before the
    attention loop; the V projection and d=1,2 projections are emitted
    as fillers inside pair 0/1's kt blocks (PE slack), so pair-0
    consumption of rel strips starts ~25 us in and the 20-deep strip
    pool never backs up the DMA queue.
  * PV accumulation steps are interleaved per kt block (skip_group_
    check), so the attention tail after the last strip arrives is only
    the last block's drain + finalize.
  * scores^T layout keeps the mask as a per-partition ACT bias: masked
    keys get FLT_MIN so exp underflows to exactly 0 (no row-max needed,
    scores are O(10)).

Per-core math (identical to reference up to fp16 rounding):
  Q^T = (Wq_s @ X^T + bq)/8, K^T = Wk_s @ X^T + bk (fp16 matmuls, fp32
  PSUM), V = X @ Wv_s^T + bv stored fp16 with a ones column (col 64 ->
  softmax denominator for free).  ps[k,q] = K^T.T@Q^T (+rel12 via DVE),
  pT = exp(ps + maskbias), ctx^T[d|1, q] += V_aug[kt].T @ pT[kt],
  out[q, h*64+d] = ctx[q, d] / ctx[q, 64].
"""

import os
import sys

import numpy as np

for _p in ("/opt/trn_rl_repo",):
    if _p not in sys.path and os.path.isdir(_p):
        sys.path.append(_p)

import concourse.bass as bass
import concourse.mybir as mybir
import concourse.tile as tile
from concourse import bacc
from concourse.bass_utils import run_bass_kernel_spmd
from concourse.masks import make_identity

F32 = mybir.dt.float32
F16 = mybir.dt.float16
I32 = mybir.dt.int32
AF = mybir.ActivationFunctionType
NEG = float(np.finfo(np.float32).min)

P = 128
S = 1024
NH = 6        # heads per core
HD = 64
HIN = 768     # model dim (contraction for projections)
HOUT = NH * HD  # 384, per-core projection width
KT = S // P   # 8 key tiles
QT = S // P   # 8 query tiles
VW = HD + 1   # 65: V columns + ones column
NPAIR = NH // 2

# 'gpsimd': GPSIMD pre-sums rel1+rel2, DVE does 1 RMW per score block.
# 'none':   DVE does 2 RMWs per score block (no pre-sum).
PRESUM = os.environ.get("K_PRESUM", "gpsimd")


def _build_kernel_body(tc, aps):
    import contextlib

    nc = tc.nc
    x_ap = aps["x"]
    mask_ap = aps["mask"]
    rel1_ap = aps["rel1"]  # [NH, S(k), S(q)] -- host-transposed
    rel2_ap = aps["rel2"]
    out_ap = aps["out"]

    with contextlib.ExitStack() as ctx:
        const = ctx.enter_context(tc.tile_pool(name="const", bufs=1))

        ident = const.tile([P, P], F16)
        make_identity(nc, ident)
        ident32 = const.tile([P, P], F32)
        nc.vector.tensor_copy(ident32[:], ident[:])

        # long-lived tensors
        qt_pool = ctx.enter_context(tc.tile_pool(name="qT", bufs=3))
        kt_pool = ctx.enter_context(tc.tile_pool(name="kT", bufs=3))
        v_pool = ctx.enter_context(tc.tile_pool(name="v", bufs=8))
        xt_pool = ctx.enter_context(tc.tile_pool(name="xT", bufs=6))
        wt_pool = ctx.enter_context(tc.tile_pool(name="wT", bufs=18))

        qT = [qt_pool.tile([P, S], F16, tag="qT", name=f"qT{i}") for i in range(3)]
        kT = [kt_pool.tile([P, S], F16, tag="kT", name=f"kT{i}") for i in range(3)]
        v_tiles = [
            v_pool.tile([P, NH, VW], F16, tag="v", name=f"v{i}") for i in range(8)
        ]

        # rel strip pool: strip DMAs queue behind the x/W loads and then
        # stream continuously for the rest of the kernel.
        r_pool = ctx.enter_context(tc.tile_pool(name="rel", bufs=20))

        # unified PSUM pools: "bigps" carries every 1-bank use (X/W
        # transpose staging, projection groups, score tiles, finalize
        # back-transposes); "vpsum" carries the 4 ctx^T accumulators.
        bigps = ctx.enter_context(tc.tile_pool(name="bigps", bufs=4, space="PSUM"))
        vpsum = ctx.enter_context(tc.tile_pool(name="vpsum", bufs=4, space="PSUM"))

        # ---------------- phase 1a: load, cast, transpose, d0 projections --
        ph1 = contextlib.ExitStack()  # closed after the last filler is used
        xload = ph1.enter_context(tc.tile_pool(name="xload", bufs=2))
        wload = ph1.enter_context(tc.tile_pool(name="wload", bufs=2))
        x16_pool = ph1.enter_context(tc.tile_pool(name="x16", bufs=8))
        w16_pool = ph1.enter_context(tc.tile_pool(name="w16", bufs=4))

        # X tiles [128, 768] -> fp16
        x16 = []
        for t in range(8):
            xt_ = xload.tile([P, HIN], F32, tag="x")
            nc.sync.dma_start(xt_[:], x_ap[t * P:(t + 1) * P, :])
            x16_t = x16_pool.tile([P, HIN], F16, tag="x16", name=f"x16_{t}")
            nc.vector.tensor_copy(x16_t[:], xt_[:])
            x16.append(x16_t)

        # W loads+casts for all three weights (Wv last)
        w16 = {}
        for wname in ("q", "k", "v"):
            w_ap = aps[f"w{wname}"]
            for d in range(3):
                wt_ = wload.tile([P, HIN], F32, tag="wload")
                nc.sync.dma_start(wt_[:], w_ap[d * P:(d + 1) * P, :])
                w16_t = w16_pool.tile(
                    [P, HIN], F16, tag="w16", name=f"w16{wname}_{d}"
                )
                nc.vector.tensor_copy(w16_t[:], wt_[:])
                w16[(wname, d)] = w16_t

        # mask bias and projection biases (off the startup critical path)
        mask_i = const.tile([P, KT], I32)
        nc.sync.dma_start(mask_i[:], mask_ap.rearrange("(a p) -> p a", p=P))
        maskb = const.tile([P, KT], F32)
        nc.vector.tensor_copy(maskb[:], mask_i[:])
        nc.vector.tensor_scalar_mul(maskb[:], maskb[:], NEG)
        bias_sb = {}
        for wname in ("q", "k"):
            bt = const.tile([P, 3], F32, tag=f"b{wname}")
            nc.sync.dma_start(
                bt[:], aps[f"b{wname}"].rearrange("(a p) -> p a", p=P)
            )
            if wname == "q":
                nc.vector.tensor_scalar_mul(bt[:], bt[:], 0.125)
            bias_sb[wname] = bt
        bv_bc = const.tile([P, NH, HD], F32)
        nc.sync.dma_start(
            bv_bc[:],
            aps["bv"].rearrange("(h d) -> h d", d=HD)[None].to_broadcast(
                (P, NH, HD)
            ),
        )

        # X^T: 6 fp16 tiles [128, 1024] (h-chunk on partitions)
        xT = []
        for hc in range(6):
            pt = bigps.tile([P, S], F16, tag="ps")  # 1 bank (fp16)
            for t in range(8):
                nc.tensor.transpose(
                    pt[:, t * P:(t + 1) * P],
                    x16[t][:, hc * P:(hc + 1) * P],
                    ident[:],
                )
            xt_t = xt_pool.tile([P, S], F16, tag="xT")
            nc.scalar.copy(xt_t[:], pt[:])
            xT.append(xt_t)

        # W^T slices (fp16): wT[(w, hc)] = [128, 384]
        wT = {}
        for wname in ("q", "k", "v"):
            for hc in range(6):
                pw = bigps.tile([P, 512], F16, tag="ps", name="pw")[:, :HOUT]
                for d in range(3):
                    nc.tensor.transpose(
                        pw[:, d * P:(d + 1) * P],
                        w16[(wname, d)][:, hc * P:(hc + 1) * P],
                        ident[:],
                    )
                wt_t = wt_pool.tile([P, HOUT], F16, tag="wT")
                nc.scalar.copy(wt_t[:], pw[:])
                wT[(wname, hc)] = wt_t

        def emit_qk_proj(wname, d, tch):
            dest = qT if wname == "q" else kT
            scale = 0.125 if wname == "q" else 1.0
            pp = bigps.tile([P, 512], F32, tag="ps")
            for hc in range(6):
                nc.tensor.matmul(
                    pp[:],
                    wT[(wname, hc)][:, d * P:(d + 1) * P],
                    xT[hc][:, tch * 512:(tch + 1) * 512],
                    start=(hc == 0),
                    stop=(hc == 5),
                )
            nc.scalar.activation(
                dest[d][:, tch * 512:(tch + 1) * 512],
                pp[:],
                AF.Identity,
                bias=bias_sb[wname][:, d:d + 1],
                scale=scale,
            )

        def emit_v_proj(t):
            pv = bigps.tile([P, 512], F32, tag="ps", name="pv")[:, :HOUT]
            for hc in range(6):
                nc.tensor.matmul(
                    pv[:],
                    xT[hc][:, t * P:(t + 1) * P],
                    wT[("v", hc)][:],
                    start=(hc == 0),
                    stop=(hc == 5),
                )
            nc.vector.memset(v_tiles[t][:], 1.0)
            nc.vector.tensor_add(
                v_tiles[t][:, :, 0:HD],
                pv[:].rearrange("p (h d) -> p h d", d=HD),
                bv_bc[:],
            )

        # d=0 projections (pair 0's heads) + V tile 0 up front; the rest
        # are fillers emitted inside pair 0/1's kt blocks.
        for wname in ("q", "k"):
            for tch in range(2):
                emit_qk_proj(wname, 0, tch)
        emit_v_proj(0)

        # fillers[dt][kt] -> list of closures to emit at that block
        fillers = [[[] for _ in range(KT)] for _ in range(NPAIR)]
        for t in range(1, 8):  # V tile t needed by pair-0 block kt=t
            fillers[0][t - 1].append(lambda t=t: emit_v_proj(t))
        for i, (wname, tch) in enumerate(
            (w, t) for w in ("q", "k") for t in range(2)
        ):
            fillers[0][2 * i].append(
                lambda w=wname, t=tch: emit_qk_proj(w, 1, t)
            )
            fillers[1][2 * i].append(
                lambda w=wname, t=tch: emit_qk_proj(w, 2, t)
            )

        # transient load/cast pools are only read by the phase-1a
        # transposes; free their SBUF for the phase-2 pools
        ph1.close()

        # ---------------- phase 2: attention per head pair -----------------
        out_pool = ctx.enter_context(tc.tile_pool(name="outst", bufs=8))
        out_stage = [
            out_pool.tile([P, HOUT], F32, tag="outst", name=f"outst{i}")
            for i in range(8)
        ]
        pt_pool = ctx.enter_context(tc.tile_pool(name="pT", bufs=8))
        fin_pool = ctx.enter_context(tc.tile_pool(name="fin", bufs=4))
        ctt_pool = ctx.enter_context(tc.tile_pool(name="ctt", bufs=4))

        def emit_fin_copy(fin):
            """ACT-copy the previous pair's ctx^T accumulators out of PSUM
            (releases the vpsum banks for this pair's PV groups)."""
            dt, ctxT_ps = fin
            ctxT_sb = {}
            for h2 in range(2):
                for qch in range(2):
                    t_ = ctt_pool.tile(
                        [VW, 512], F32, tag="ctxT_sb", name=f"ctT{dt}_{h2}_{qch}"
                    )
                    nc.scalar.copy(t_[:], ctxT_ps[(h2, qch)][:])
                    ctxT_sb[(h2, qch)] = t_
            return ctxT_sb

        def emit_fin_rest(fin, ctxT_sb, emit_out_dma):
            """Back-transpose ctx^T per head, divide by the denominator,
            write out_stage (and the output DMAs for the last pair)."""
            dt, _ = fin
            for h2 in range(2):
                h = 2 * dt + h2
                ctx_ps = [
                    bigps.tile([P, 512], F32, tag="ps", name=f"ctx{h}_{i}")
                    for i in range(2)
                ]
                for qt in range(QT):
                    cp = ctx_ps[qt // 4]
                    sl = (qt % 4) * VW
                    nc.tensor.transpose(
                        cp[:, sl:sl + VW],
                        ctxT_sb[(h2, qt // 4)][:, (qt % 4) * P:(qt % 4 + 1) * P],
                        ident32[:VW, :VW],
                    )
                for qt in range(QT):
                    cp = ctx_ps[qt // 4]
                    sl = (qt % 4) * VW
                    rc = fin_pool.tile([P, 1], F32, tag="recip")
                    nc.vector.reciprocal(rc[:], cp[:, sl + HD:sl + HD + 1])
                    nc.scalar.activation(
                        out_stage[qt][:, h * HD:(h + 1) * HD],
                        cp[:, sl:sl + HD],
                        AF.Identity,
                        scale=rc[:],
                    )
                    if emit_out_dma and h2 == 1:
                        nc.sync.dma_start(
                            out_ap[qt * P:(qt + 1) * P, :], out_stage[qt][:]
                        )

        pending_fin = None
        for dt in range(NPAIR):
            # rel strips for both heads: [k=128, q=1024] fp32, kt-major,
            # heads interleaved to match consumption order.
            r1 = [[None] * KT for _ in range(2)]
            r2 = [[None] * KT for _ in range(2)]
            for kt in range(KT):
                for h2 in range(2):
                    h = 2 * dt + h2
                    t1 = r_pool.tile([P, S], F32, tag="rel", name=f"r1_{h}_{kt}")
                    nc.sync.dma_start(t1[:], rel1_ap[h][kt * P:(kt + 1) * P, :])
                    r1[h2][kt] = t1
                    t2 = r_pool.tile([P, S], F32, tag="rel", name=f"r2_{h}_{kt}")
                    nc.sync.dma_start(t2[:], rel2_ap[h][kt * P:(kt + 1) * P, :])
                    r2[h2][kt] = t2

            if pending_fin is not None:
                fin_sb = emit_fin_copy(pending_fin)

            ctxT_ps = {}
            for qch in range(2):
                for h2 in range(2):
                    ctxT_ps[(h2, qch)] = vpsum.tile(
                        [VW, 512], F32, tag="ctxT", name=f"ctxT{dt}_{h2}_{qch}"
                    )

            # kt blocks: strips fully consumed within their block; PV
            # accumulation steps interleaved so the tail after the last
            # strip is only one block's drain.
            for kt in range(KT):
                pT_kt = [
                    pt_pool.tile([P, S], F16, tag="pT", name=f"pT{dt}_{h2}_{kt}")
                    for h2 in range(2)
                ]
                if PRESUM == "gpsimd":
                    for h2 in range(2):
                        nc.gpsimd.tensor_add(
                            r1[h2][kt][:], r1[h2][kt][:], r2[h2][kt][:]
                        )
                ps4 = {}
                for qch in range(2):
                    qsl = slice(qch * 512, (qch + 1) * 512)
                    for h2 in range(2):
                        d0 = h2 * HD
                        ps = bigps.tile([P, 512], F32, tag="ps")
                        # back-to-back K=64 matmuls at base partitions 0/64
                        # get distinct row-group tile_positions -> run
                        # concurrently on the PE array
                        nc.tensor.matmul(
                            ps[:],
                            kT[dt][d0:d0 + HD, kt * P:(kt + 1) * P],
                            qT[dt][d0:d0 + HD, qsl],
                            start=True,
                            stop=True,
                        )
                        ps4[(qch, h2)] = ps
                for f in fillers[dt][kt]:
                    f()
                for qch in range(2):
                    qsl = slice(qch * 512, (qch + 1) * 512)
                    for h2 in range(2):
                        ps = ps4[(qch, h2)]
                        nc.vector.tensor_add(ps[:], ps[:], r1[h2][kt][:, qsl])
                        if PRESUM != "gpsimd":
                            nc.vector.tensor_add(
                                ps[:], ps[:], r2[h2][kt][:, qsl]
                            )
                        nc.scalar.activation(
                            pT_kt[h2][:, qsl],
                            ps[:],
                            AF.Exp,
                            bias=maskb[:, kt:kt + 1],
                            scale=1.0,
                        )
                # PV steps for this kt (both heads x both q-chunks)
                for qch in range(2):
                    qsl = slice(qch * 512, (qch + 1) * 512)
                    for h2 in range(2):
                        h = 2 * dt + h2
                        nc.tensor.matmul(
                            ctxT_ps[(h2, qch)][:],
                            v_tiles[kt][:, h, :],
                            pT_kt[h2][:, qsl],
                            start=(kt == 0),
                            stop=(kt == KT - 1),
                            skip_group_check=True,
                        )

            if pending_fin is not None:
                emit_fin_rest(pending_fin, fin_sb, emit_out_dma=False)
            pending_fin = (dt, ctxT_ps)

        fin_sb = emit_fin_copy(pending_fin)
        emit_fin_rest(pending_fin, fin_sb, emit_out_dma=True)


def build_program():
    """Build and compile the per-core Bass program. Returns nc."""
    nc = bacc.Bacc(
        "TRN2",
        target_bir_lowering=False,
        debug=False,
        num_devices=8,
    )
    aps = {
        "x": nc.dram_tensor("x", [S, HIN], F32, kind="ExternalInput").ap(),
        "mask": nc.dram_tensor("mask", [S], I32, kind="ExternalInput").ap(),
        "rel1": nc.dram_tensor("rel1", [NH, S, S], F32, kind="ExternalInput").ap(),
        "rel2": nc.dram_tensor("rel2", [NH, S, S], F32, kind="ExternalInput").ap(),
        "wq": nc.dram_tensor("wq", [HOUT, HIN], F32, kind="ExternalInput").ap(),
        "wk": nc.dram_tensor("wk", [HOUT, HIN], F32, kind="ExternalInput").ap(),
        "wv": nc.dram_tensor("wv", [HOUT, HIN], F32, kind="ExternalInput").ap(),
        "bq": nc.dram_tensor("bq", [HOUT], F32, kind="ExternalInput").ap(),
        "bk": nc.dram_tensor("bk", [HOUT], F32, kind="ExternalInput").ap(),
        "bv": nc.dram_tensor("bv", [HOUT], F32, kind="ExternalInput").ap(),
        "out": nc.dram_tensor("out", [S, HOUT], F32, kind="ExternalOutput").ap(),
    }
    with tile.TileContext(nc) as tc:
        _build_kernel_body(tc, aps)
    nc.compile()
    return nc


def make_in_maps(inputs):
    """Slice full inputs into the 8 per-core input maps.

    rel_pos / rel_2d_pos are uploaded transposed per head ([k, q] layout)
    so their strips add directly into the transposed score tiles."""
    hs = np.ascontiguousarray(np.asarray(inputs["hidden_states"], np.float32))
    am = np.asarray(inputs["attention_mask"]).astype(np.int32)
    rel1 = np.asarray(inputs["rel_pos"], np.float32)
    rel2 = np.asarray(inputs["rel_2d_pos"], np.float32)
    ws = {k: np.asarray(inputs["W" + k[-1]], np.float32) for k in ("wq", "wk", "wv")}
    bs = {k: np.asarray(inputs["b" + k[-1]], np.float32) for k in ("bq", "bk", "bv")}

    rel1T = np.ascontiguousarray(rel1.transpose(0, 1, 3, 2))
    rel2T = np.ascontiguousarray(rel2.transpose(0, 1, 3, 2))

    in_maps = []
    for c in range(8):
        b, hh = divmod(c, 2)
        hsl = slice(hh * NH, (hh + 1) * NH)
        csl = slice(hh * HOUT, (hh + 1) * HOUT)
        m = {
            "x": np.ascontiguousarray(hs[b]),
            "mask": np.ascontiguousarray(am[b, 0, 0]),
            "rel1": np.ascontiguousarray(rel1T[b, hsl]),
            "rel2": np.ascontiguousarray(rel2T[b, hsl]),
        }
        for k in ("wq", "wk", "wv"):
            m[k] = np.ascontiguousarray(ws[k][csl])
        for k in ("bq", "bk", "bv"):
            m[k] = np.ascontiguousarray(bs[k][csl])
        in_maps.append(m)
    return in_maps


def gather_output(results):
    out = np.empty((4, S, HIN), np.float32)
    for c in range(8):
        b, hh = divmod(c, 2)
        out[b, :, hh * HOUT:(hh + 1) * HOUT] = results[c]["out"]
    return out


_NC_CACHE = []


def kernel(**inputs):
    if not _NC_CACHE:
        _NC_CACHE.append(build_program())
    nc = _NC_CACHE[0]
    in_maps = make_in_maps(inputs)
    res = run_bass_kernel_spmd(nc, in_maps, list(range(8)))
    return gather_output(res.results)


# revision 11
# speedup vs baseline: 1.1549x; 1.0100x over previous
"""ErnieLayout self-attention on 8 Trainium2 NeuronCores (Bass/Tile). v3

Problem shapes (hardcoded): B=4, S=1024, H=768, NH=12, HD=64.
Sharding: core c -> (batch b = c//2, head-half hh = c%2, i.e. 6 heads).
Each core computes attention for its 6 heads of one batch element and
writes the [S, 384] column slice of that batch's output.

The kernel is HBM-bound: rel_pos + rel_2d_pos are 50.3 MB per core of
the ~58 MB total I/O, so the design keeps the DMA queues saturated and
sizes every engine's work under the ~160 us DMA floor (robust even when
the PE is power-throttled to 1.2 GHz, which traces show happens for most
of the kernel).

Key structure:
  * rel_pos / rel_2d_pos are uploaded HOST-TRANSPOSED per head ([k, q]
    layout, a pure layout change done while sharding).  Strips land
    contiguously; GPSIMD pre-sums rel1+rel2 in place (idle engine), and
    the DVE adds the sum straight into the transposed score PSUM with
    one RMW per [128,512] block.  No PE transposes of rel at all.
  * heads are processed in pairs (2dt, 2dt+1) whose q/k rows live in
    partitions 0-63 / 64-127 of qT/kT tile dt: the two QK score matmuls
    per (kt, qch) are emitted back-to-back and run CONCURRENTLY on the
    PE via row tiling (auto tile_position from base partitions).
  * only the X/W transposes and the d=0 Q/K projections run before the
    attention loop; the V projection and d=1,2 projections are emitted
    as fillers inside pair 0/1's kt blocks (PE slack), so pair-0
    consumption of rel strips starts ~25 us in and the 20-deep strip
    pool never backs up the DMA queue.
  * PV accumulation steps are interleaved per kt block (skip_group_
    check), so the attention tail after the last strip arrives is only
    the last block's drain + finalize.
  * scores^T layout keeps the mask as a per-partition ACT bias: masked
    keys get FLT_MIN so exp underflows to exactly 0 (no row-max needed,
    scores are O(10)).

Per-core math (identical to reference up to fp16 rounding):
  Q^T = (Wq_s @ X^T + bq)/8, K^T = Wk_s @ X^T + bk (fp16 matmuls, fp32
  PSUM), V = X @ Wv_s^T + bv stored fp16 with a ones column (col 64 ->
  softmax denominator for free).  ps[k,q] = K^T.T@Q^T (+rel12 via DVE),
  pT = exp(ps + maskbias), ctx^T[d|1, q] += V_aug[kt].T @ pT[kt],
  out[q, h*64+d] = ctx[q, d] / ctx[q, 64].
"""

import os
import sys

import numpy as np

for _p in ("/opt/trn_rl_repo",):
    if _p not in sys.path and os.path.isdir(_p):
        sys.path.append(_p)

import concourse.bass as bass
import concourse.mybir as mybir
import concourse.tile as tile
from concourse import bacc
from concourse.bass_utils import run_bass_kernel_spmd
from concourse.masks import make_identity

F32 = mybir.dt.float32
F16 = mybir.dt.float16
I32 = mybir.dt.int32
AF = mybir.ActivationFunctionType
NEG = float(np.finfo(np.float32).min)

P = 128
S = 1024
NH = 6        # heads per core
HD = 64
HIN = 768     # model dim (contraction for projections)
HOUT = NH * HD  # 384, per-core projection width
KT = S // P   # 8 key tiles
QT = S // P   # 8 query tiles
VW = HD + 1   # 65: V columns + ones column
NPAIR = NH // 2

# 'split':  GPSIMD pre-sums rel1+rel2 for head A, DVE does 2 RMWs for
#           head B (balances the two engines under the DMA pace).
# 'gpsimd': GPSIMD pre-sums everything, DVE does 1 RMW per score block.
# 'none':   DVE does 2 RMWs per score block (no pre-sum).
PRESUM = os.environ.get("K_PRESUM", "split")
PRESUM_H2 = {"gpsimd": (True, True), "split": (True, False),
             "none": (False, False)}[PRESUM]


def _build_kernel_body(tc, aps):
    import contextlib

    nc = tc.nc
    x_ap = aps["x"]
    mask_ap = aps["mask"]
    rel1_ap = aps["rel1"]  # [NH, S(k), S(q)] -- host-transposed
    rel2_ap = aps["rel2"]
    out_ap = aps["out"]

    with contextlib.ExitStack() as ctx:
        const = ctx.enter_context(tc.tile_pool(name="const", bufs=1))

        ident = const.tile([P, P], F16)
        make_identity(nc, ident)
        ident32 = const.tile([P, P], F32)
        nc.vector.tensor_copy(ident32[:], ident[:])

        # long-lived tensors
        qt_pool = ctx.enter_context(tc.tile_pool(name="qT", bufs=3))
        kt_pool = ctx.enter_context(tc.tile_pool(name="kT", bufs=3))
        v_pool = ctx.enter_context(tc.tile_pool(name="v", bufs=8))
        xt_pool = ctx.enter_context(tc.tile_pool(name="xT", bufs=6))
        wt_pool = ctx.enter_context(tc.tile_pool(name="wT", bufs=18))

        qT = [qt_pool.tile([P, S], F16, tag="qT", name=f"qT{i}") for i in range(3)]
        kT = [kt_pool.tile([P, S], F16, tag="kT", name=f"kT{i}") for i in range(3)]
        v_tiles = [
            v_pool.tile([P, NH, VW], F16, tag="v", name=f"v{i}") for i in range(8)
        ]

        # rel strip pool: strip DMAs queue behind the x/W loads and then
        # stream continuously for the rest of the kernel.
        r_pool = ctx.enter_context(tc.tile_pool(name="rel", bufs=20))

        # unified PSUM pools: "bigps" carries every 1-bank use (X/W
        # transpose staging, projection groups, score tiles, finalize
        # back-transposes); "vpsum" carries the 4 ctx^T accumulators.
        bigps = ctx.enter_context(tc.tile_pool(name="bigps", bufs=4, space="PSUM"))
        vpsum = ctx.enter_context(tc.tile_pool(name="vpsum", bufs=4, space="PSUM"))

        # ---------------- phase 1a: load, cast, transpose, d0 projections --
        ph1 = contextlib.ExitStack()  # closed after the last filler is used
        xload = ph1.enter_context(tc.tile_pool(name="xload", bufs=2))
        wload = ph1.enter_context(tc.tile_pool(name="wload", bufs=2))
        x16_pool = ph1.enter_context(tc.tile_pool(name="x16", bufs=2))
        w16_pool = ph1.enter_context(tc.tile_pool(name="w16", bufs=3))

        # X tiles, two batched loads of 4 row-tiles each -> fp16
        x_r = x_ap.rearrange("(t p) c -> p t c", p=P)
        x16 = []
        for half in range(2):
            xt_ = xload.tile([P, 4, HIN], F32, tag="x")
            nc.sync.dma_start(xt_[:], x_r[:, half * 4:(half + 1) * 4, :])
            x16_b = x16_pool.tile([P, 4, HIN], F16, tag="x16",
                                  name=f"x16_{half}")
            nc.vector.tensor_copy(x16_b[:], xt_[:])
            x16.extend(x16_b[:, i, :] for i in range(4))

        # W loads+casts, one batched DMA per weight (Wv last)
        w16 = {}
        for wname in ("q", "k", "v"):
            w_r = aps[f"w{wname}"].rearrange("(d p) c -> p d c", p=P)
            wt_ = wload.tile([P, 3, HIN], F32, tag="wload")
            nc.sync.dma_start(wt_[:], w_r[:])
            w16_b = w16_pool.tile([P, 3, HIN], F16, tag="w16",
                                  name=f"w16{wname}")
            nc.vector.tensor_copy(w16_b[:], wt_[:])
            for d in range(3):
                w16[(wname, d)] = w16_b[:, d, :]

        # mask bias and projection biases (off the startup critical path)
        mask_i = const.tile([P, KT], I32)
        nc.sync.dma_start(mask_i[:], mask_ap.rearrange("(a p) -> p a", p=P))
        maskb = const.tile([P, KT], F32)
        nc.vector.tensor_copy(maskb[:], mask_i[:])
        nc.vector.tensor_scalar_mul(maskb[:], maskb[:], NEG)
        bias_sb = {}
        for wname in ("q", "k"):
            bt = const.tile([P, 3], F32, tag=f"b{wname}")
            nc.sync.dma_start(
                bt[:], aps[f"b{wname}"].rearrange("(a p) -> p a", p=P)
            )
            if wname == "q":
                nc.vector.tensor_scalar_mul(bt[:], bt[:], 0.125)
            bias_sb[wname] = bt
        bv_bc = const.tile([P, NH, HD], F32)
        nc.sync.dma_start(
            bv_bc[:],
            aps["bv"].rearrange("(h d) -> h d", d=HD)[None].to_broadcast(
                (P, NH, HD)
            ),
        )

        # X^T: 6 fp16 tiles [128, 1024] (h-chunk on partitions)
        xT = []
        for hc in range(6):
            pt = bigps.tile([P, S], F16, tag="ps")  # 1 bank (fp16)
            for t in range(8):
                nc.tensor.transpose(
                    pt[:, t * P:(t + 1) * P],
                    x16[t][:, hc * P:(hc + 1) * P],
                    ident[:],
                )
            xt_t = xt_pool.tile([P, S], F16, tag="xT")
            nc.scalar.copy(xt_t[:], pt[:])
            xT.append(xt_t)

        # W^T slices (fp16): wT[(w, hc)] = [128, 384]
        wT = {}
        for wname in ("q", "k", "v"):
            for hc in range(6):
                pw = bigps.tile([P, 512], F16, tag="ps", name="pw")[:, :HOUT]
                for d in range(3):
                    nc.tensor.transpose(
                        pw[:, d * P:(d + 1) * P],
                        w16[(wname, d)][:, hc * P:(hc + 1) * P],
                        ident[:],
                    )
                wt_t = wt_pool.tile([P, HOUT], F16, tag="wT")
                nc.scalar.copy(wt_t[:], pw[:])
                wT[(wname, hc)] = wt_t

        def emit_qk_proj(wname, d, tch):
            dest = qT if wname == "q" else kT
            scale = 0.125 if wname == "q" else 1.0
            pp = bigps.tile([P, 512], F32, tag="ps")
            for hc in range(6):
                nc.tensor.matmul(
                    pp[:],
                    wT[(wname, hc)][:, d * P:(d + 1) * P],
                    xT[hc][:, tch * 512:(tch + 1) * 512],
                    start=(hc == 0),
                    stop=(hc == 5),
                )
            nc.scalar.activation(
                dest[d][:, tch * 512:(tch + 1) * 512],
                pp[:],
                AF.Identity,
                bias=bias_sb[wname][:, d:d + 1],
                scale=scale,
            )

        def emit_v_proj(t):
            pv = bigps.tile([P, 512], F32, tag="ps", name="pv")[:, :HOUT]
            for hc in range(6):
                nc.tensor.matmul(
                    pv[:],
                    xT[hc][:, t * P:(t + 1) * P],
                    wT[("v", hc)][:],
                    start=(hc == 0),
                    stop=(hc == 5),
                )
            nc.vector.memset(v_tiles[t][:], 1.0)
            nc.vector.tensor_add(
                v_tiles[t][:, :, 0:HD],
                pv[:].rearrange("p (h d) -> p h d", d=HD),
                bv_bc[:],
            )

        # d=0 projections (pair 0's heads) + V tile 0 up front; the rest
        # are fillers emitted inside pair 0/1's kt blocks.
        for wname in ("q", "k"):
            for tch in range(2):
                emit_qk_proj(wname, 0, tch)
        emit_v_proj(0)

        # fillers[dt][kt] -> list of closures to emit at that block
        fillers = [[[] for _ in range(KT)] for _ in range(NPAIR)]
        for t in range(1, 8):  # V tile t needed by pair-0 block kt=t
            fillers[0][t - 1].append(lambda t=t: emit_v_proj(t))
        for i, (wname, tch) in enumerate(
            (w, t) for w in ("q", "k") for t in range(2)
        ):
            fillers[0][2 * i].append(
                lambda w=wname, t=tch: emit_qk_proj(w, 1, t)
            )
            fillers[1][2 * i].append(
                lambda w=wname, t=tch: emit_qk_proj(w, 2, t)
            )

        # transient load/cast pools are only read by the phase-1a
        # transposes; free their SBUF for the phase-2 pools
        ph1.close()

        # ---------------- phase 2: attention per head pair -----------------
        out_pool = ctx.enter_context(tc.tile_pool(name="outst", bufs=8))
        out_stage = [
            out_pool.tile([P, HOUT], F32, tag="outst", name=f"outst{i}")
            for i in range(8)
        ]
        pt_pool = ctx.enter_context(tc.tile_pool(name="pT", bufs=8))
        fin_pool = ctx.enter_context(tc.tile_pool(name="fin", bufs=4))
        ctt_pool = ctx.enter_context(tc.tile_pool(name="ctt", bufs=4))

        def emit_fin_copy(fin, ctxT_sb, h2s=(0, 1)):
            """ACT-copy the previous pair's ctx^T accumulators out of PSUM
            (releases the vpsum banks for this pair's PV groups)."""
            dt, ctxT_ps = fin
            for h2 in h2s:
                for qch in range(2):
                    t_ = ctt_pool.tile(
                        [VW, 512], F32, tag="ctxT_sb", name=f"ctT{dt}_{h2}_{qch}"
                    )
                    nc.scalar.copy(t_[:], ctxT_ps[(h2, qch)][:])
                    ctxT_sb[(h2, qch)] = t_
            return ctxT_sb

        def emit_fin_rest(fin, ctxT_sb, h2s, emit_out_dma):
            """Back-transpose ctx^T per head, divide by the denominator,
            write out_stage (and the output DMAs for the last pair)."""
            dt, _ = fin
            for h2 in h2s:
                h = 2 * dt + h2
                ctx_ps = [
                    bigps.tile([P, 512], F32, tag="ps", name=f"ctx{h}_{i}")
                    for i in range(2)
                ]
                for qt in range(QT):
                    cp = ctx_ps[qt // 4]
                    sl = (qt % 4) * VW
                    nc.tensor.transpose(
                        cp[:, sl:sl + VW],
                        ctxT_sb[(h2, qt // 4)][:, (qt % 4) * P:(qt % 4 + 1) * P],
                        ident32[:VW, :VW],
                    )
                for qt in range(QT):
                    cp = ctx_ps[qt // 4]
                    sl = (qt % 4) * VW
                    rc = fin_pool.tile([P, 1], F32, tag="recip")
                    nc.vector.reciprocal(rc[:], cp[:, sl + HD:sl + HD + 1])
                    nc.scalar.activation(
                        out_stage[qt][:, h * HD:(h + 1) * HD],
                        cp[:, sl:sl + HD],
                        AF.Identity,
                        scale=rc[:],
                    )
                    if emit_out_dma and h2 == 1:
                        nc.sync.dma_start(
                            out_ap[qt * P:(qt + 1) * P, :], out_stage[qt][:]
                        )

        pending_fin = None
        for dt in range(NPAIR):
            # rel strips for both heads: [k=128, q=1024] fp32, kt-major,
            # heads interleaved to match consumption order.
            r1 = [[None] * KT for _ in range(2)]
            r2 = [[None] * KT for _ in range(2)]
            for kt in range(KT):
                for h2 in range(2):
                    h = 2 * dt + h2
                    t1 = r_pool.tile([P, S], F32, tag="rel", name=f"r1_{h}_{kt}")
                    nc.sync.dma_start(t1[:], rel1_ap[h][kt * P:(kt + 1) * P, :])
                    r1[h2][kt] = t1
                    t2 = r_pool.tile([P, S], F32, tag="rel", name=f"r2_{h}_{kt}")
                    nc.sync.dma_start(t2[:], rel2_ap[h][kt * P:(kt + 1) * P, :])
                    r2[h2][kt] = t2

            if pending_fin is not None:
                fin_sb = {}
                emit_fin_copy(pending_fin, fin_sb)

            ctxT_ps = {}
            for qch in range(2):
                for h2 in range(2):
                    ctxT_ps[(h2, qch)] = vpsum.tile(
                        [VW, 512], F32, tag="ctxT", name=f"ctxT{dt}_{h2}_{qch}"
                    )

            # kt blocks: strips fully consumed within their block; PV
            # accumulation steps interleaved so the tail after the last
            # strip is only one block's drain.
            for kt in range(KT):
                pT_kt = [
                    pt_pool.tile([P, S], F16, tag="pT", name=f"pT{dt}_{h2}_{kt}")
                    for h2 in range(2)
                ]
                for h2 in range(2):
                    if PRESUM_H2[h2]:
                        nc.gpsimd.tensor_add(
                            r1[h2][kt][:], r1[h2][kt][:], r2[h2][kt][:]
                        )
                ps4 = {}
                for qch in range(2):
                    qsl = slice(qch * 512, (qch + 1) * 512)
                    for h2 in range(2):
                        d0 = h2 * HD
                        ps = bigps.tile([P, 512], F32, tag="ps")
                        # back-to-back K=64 matmuls at base partitions 0/64
                        # get distinct row-group tile_positions -> run
                        # concurrently on the PE array
                        nc.tensor.matmul(
                            ps[:],
                            kT[dt][d0:d0 + HD, kt * P:(kt + 1) * P],
                            qT[dt][d0:d0 + HD, qsl],
                            start=True,
                            stop=True,
                        )
                        ps4[(qch, h2)] = ps
                for f in fillers[dt][kt]:
                    f()
                for qch in range(2):
                    qsl = slice(qch * 512, (qch + 1) * 512)
                    for h2 in range(2):
                        ps = ps4[(qch, h2)]
                        nc.vector.tensor_add(ps[:], ps[:], r1[h2][kt][:, qsl])
                        if not PRESUM_H2[h2]:
                            nc.vector.tensor_add(
                                ps[:], ps[:], r2[h2][kt][:, qsl]
                            )
                        nc.scalar.activation(
                            pT_kt[h2][:, qsl],
                            ps[:],
                            AF.Exp,
                            bias=maskb[:, kt:kt + 1],
                            scale=1.0,
                        )
                # PV steps for this kt (both heads x both q-chunks)
                for qch in range(2):
                    qsl = slice(qch * 512, (qch + 1) * 512)
                    for h2 in range(2):
                        h = 2 * dt + h2
                        nc.tensor.matmul(
                            ctxT_ps[(h2, qch)][:],
                            v_tiles[kt][:, h, :],
                            pT_kt[h2][:, qsl],
                            start=(kt == 0),
                            stop=(kt == KT - 1),
                            skip_group_check=True,
                        )
                # previous pair's finalize, spread mid-pair so it never
                # lands in the post-DMA tail
                if pending_fin is not None and kt in (2, 5):
                    emit_fin_rest(pending_fin, fin_sb, (kt // 3,),
                                  emit_out_dma=False)

            pending_fin = (dt, ctxT_ps)

        # last pair: per-head finalize immediately, head A first
        fin_sb = {}
        for h2 in range(2):
            emit_fin_copy(pending_fin, fin_sb, (h2,))
            emit_fin_rest(pending_fin, fin_sb, (h2,), emit_out_dma=True)


def build_program():
    """Build and compile the per-core Bass program. Returns nc."""
    nc = bacc.Bacc(
        "TRN2",
        target_bir_lowering=False,
        debug=False,
        num_devices=8,
    )
    aps = {
        "x": nc.dram_tensor("x", [S, HIN], F32, kind="ExternalInput").ap(),
        "mask": nc.dram_tensor("mask", [S], I32, kind="ExternalInput").ap(),
        "rel1": nc.dram_tensor("rel1", [NH, S, S], F32, kind="ExternalInput").ap(),
        "rel2": nc.dram_tensor("rel2", [NH, S, S], F32, kind="ExternalInput").ap(),
        "wq": nc.dram_tensor("wq", [HOUT, HIN], F32, kind="ExternalInput").ap(),
        "wk": nc.dram_tensor("wk", [HOUT, HIN], F32, kind="ExternalInput").ap(),
        "wv": nc.dram_tensor("wv", [HOUT, HIN], F32, kind="ExternalInput").ap(),
        "bq": nc.dram_tensor("bq", [HOUT], F32, kind="ExternalInput").ap(),
        "bk": nc.dram_tensor("bk", [HOUT], F32, kind="ExternalInput").ap(),
        "bv": nc.dram_tensor("bv", [HOUT], F32, kind="ExternalInput").ap(),
        "out": nc.dram_tensor("out", [S, HOUT], F32, kind="ExternalOutput").ap(),
    }
    with tile.TileContext(nc) as tc:
        _build_kernel_body(tc, aps)
    nc.compile()
    return nc


def make_in_maps(inputs):
    """Slice full inputs into the 8 per-core input maps.

    rel_pos / rel_2d_pos are uploaded transposed per head ([k, q] layout)
    so their strips add directly into the transposed score tiles."""
    hs = np.ascontiguousarray(np.asarray(inputs["hidden_states"], np.float32))
    am = np.asarray(inputs["attention_mask"]).astype(np.int32)
    rel1 = np.asarray(inputs["rel_pos"], np.float32)
    rel2 = np.asarray(inputs["rel_2d_pos"], np.float32)
    ws = {k: np.asarray(inputs["W" + k[-1]], np.float32) for k in ("wq", "wk", "wv")}
    bs = {k: np.asarray(inputs["b" + k[-1]], np.float32) for k in ("bq", "bk", "bv")}

    rel1T = np.ascontiguousarray(rel1.transpose(0, 1, 3, 2))
    rel2T = np.ascontiguousarray(rel2.transpose(0, 1, 3, 2))

    in_maps = []
    for c in range(8):
        b, hh = divmod(c, 2)
        hsl = slice(hh * NH, (hh + 1) * NH)
        csl = slice(hh * HOUT, (hh + 1) * HOUT)
        m = {
            "x": np.ascontiguousarray(hs[b]),
            "mask": np.ascontiguousarray(am[b, 0, 0]),
            "rel1": np.ascontiguousarray(rel1T[b, hsl]),
            "rel2": np.ascontiguousarray(rel2T[b, hsl]),
        }
        for k in ("wq", "wk", "wv"):
            m[k] = np.ascontiguousarray(ws[k][csl])
        for k in ("bq", "bk", "bv"):
            m[k] = np.ascontiguousarray(bs[k][csl])
        in_maps.append(m)
    return in_maps


def gather_output(results):
    out = np.empty((4, S, HIN), np.float32)
    for c in range(8):
        b, hh = divmod(c, 2)
        out[b, :, hh * HOUT:(hh + 1) * HOUT] = results[c]["out"]
    return out


_NC_CACHE = []


def kernel(**inputs):
    if not _NC_CACHE:
        _NC_CACHE.append(build_program())
    nc = _NC_CACHE[0]
    in_maps = make_in_maps(inputs)
    res = run_bass_kernel_spmd(nc, in_maps, list(range(8)))
    return gather_output(res.results)


# revision 12
# speedup vs baseline: 1.2452x; 1.0781x over previous
"""ErnieLayout self-attention on 8 Trainium2 NeuronCores (Bass/Tile). v3

Problem shapes (hardcoded): B=4, S=1024, H=768, NH=12, HD=64.
Sharding: core c -> (batch b = c//2, head-half hh = c%2, i.e. 6 heads).
Each core computes attention for its 6 heads of one batch element and
writes the [S, 384] column slice of that batch's output.

The kernel is HBM-bound: rel_pos + rel_2d_pos are 50.3 MB per core of
the ~58 MB total I/O, so the design keeps the DMA queues saturated and
sizes every engine's work under the ~160 us DMA floor (robust even when
the PE is power-throttled to 1.2 GHz, which traces show happens for most
of the kernel).

Key structure:
  * rel_pos / rel_2d_pos are uploaded HOST-TRANSPOSED per head ([k, q]
    layout, a pure layout change done while sharding).  Strips land
    contiguously; GPSIMD pre-sums rel1+rel2 in place (idle engine), and
    the DVE adds the sum straight into the transposed score PSUM with
    one RMW per [128,512] block.  No PE transposes of rel at all.
  * heads are processed in pairs (2dt, 2dt+1) whose q/k rows live in
    partitions 0-63 / 64-127 of qT/kT tile dt: the two QK score matmuls
    per (kt, qch) are emitted back-to-back and run CONCURRENTLY on the
    PE via row tiling (auto tile_position from base partitions).
  * only the X/W transposes and the d=0 Q/K projections run before the
    attention loop; the V projection and d=1,2 projections are emitted
    as fillers inside pair 0/1's kt blocks (PE slack), so pair-0
    consumption of rel strips starts ~25 us in and the 20-deep strip
    pool never backs up the DMA queue.
  * PV accumulation steps are interleaved per kt block (skip_group_
    check), so the attention tail after the last strip arrives is only
    the last block's drain + finalize.
  * scores^T layout keeps the mask as a per-partition ACT bias: masked
    keys get FLT_MIN so exp underflows to exactly 0 (no row-max needed,
    scores are O(10)).

Per-core math (identical to reference up to fp16 rounding):
  Q^T = (Wq_s @ X^T + bq)/8, K^T = Wk_s @ X^T + bk (fp16 matmuls, fp32
  PSUM), V = X @ Wv_s^T + bv stored fp16 with a ones column (col 64 ->
  softmax denominator for free).  ps[k,q] = K^T.T@Q^T (+rel12 via DVE),
  pT = exp(ps + maskbias), ctx^T[d|1, q] += V_aug[kt].T @ pT[kt],
  out[q, h*64+d] = ctx[q, d] / ctx[q, 64].
"""

import os
import sys

import numpy as np

for _p in ("/opt/trn_rl_repo",):
    if _p not in sys.path and os.path.isdir(_p):
        sys.path.append(_p)

import concourse.bass as bass
import concourse.mybir as mybir
import concourse.tile as tile
from concourse import bacc
from concourse.bass_utils import run_bass_kernel_spmd
from concourse.masks import make_identity

F32 = mybir.dt.float32
F16 = mybir.dt.float16
I32 = mybir.dt.int32
AF = mybir.ActivationFunctionType
NEG = float(np.finfo(np.float32).min)

P = 128
S = 1024
NH = 6        # heads per core
HD = 64
HIN = 768     # model dim (contraction for projections)
HOUT = NH * HD  # 384, per-core projection width
KT = S // P   # 8 key tiles
QT = S // P   # 8 query tiles
VW = HD + 1   # 65: V columns + ones column
NPAIR = NH // 2

# 'split':  GPSIMD pre-sums rel1+rel2 for head A, DVE does 2 RMWs for
#           head B (balances the two engines under the DMA pace).
# 'gpsimd': GPSIMD pre-sums everything, DVE does 1 RMW per score block.
# 'none':   DVE does 2 RMWs per score block (no pre-sum).
PRESUM = os.environ.get("K_PRESUM", "split")
PRESUM_H2 = {"gpsimd": (True, True), "split": (True, False),
             "none": (False, False)}[PRESUM]


def _build_kernel_body(tc, aps):
    import contextlib

    nc = tc.nc
    x_ap = aps["x"]
    mask_ap = aps["mask"]
    rel1_ap = aps["rel1"]  # [NH, S(k), S(q)] -- host-transposed
    rel2_ap = aps["rel2"]
    out_ap = aps["out"]

    with contextlib.ExitStack() as ctx:
        const = ctx.enter_context(tc.tile_pool(name="const", bufs=1))

        ident = const.tile([P, P], F16)
        make_identity(nc, ident)
        ident32 = const.tile([P, P], F32)
        nc.vector.tensor_copy(ident32[:], ident[:])

        # long-lived tensors
        qt_pool = ctx.enter_context(tc.tile_pool(name="qT", bufs=3))
        kt_pool = ctx.enter_context(tc.tile_pool(name="kT", bufs=3))
        v_pool = ctx.enter_context(tc.tile_pool(name="v", bufs=8))
        xt_pool = ctx.enter_context(tc.tile_pool(name="xT", bufs=6))
        wt_pool = ctx.enter_context(tc.tile_pool(name="wT", bufs=18))

        qT = [qt_pool.tile([P, S], F16, tag="qT", name=f"qT{i}") for i in range(3)]
        kT = [kt_pool.tile([P, S], F16, tag="kT", name=f"kT{i}") for i in range(3)]
        v_tiles = [
            v_pool.tile([P, NH, VW], F16, tag="v", name=f"v{i}") for i in range(8)
        ]

        # rel strip pool: strip DMAs queue behind the x/W loads and then
        # stream continuously for the rest of the kernel.
        r_pool = ctx.enter_context(tc.tile_pool(name="rel", bufs=22))

        # unified PSUM pools: "bigps" carries every 1-bank use (X/W
        # transpose staging, projection groups, score tiles, finalize
        # back-transposes); "vpsum" carries the 4 ctx^T accumulators.
        bigps = ctx.enter_context(tc.tile_pool(name="bigps", bufs=4, space="PSUM"))
        vpsum = ctx.enter_context(tc.tile_pool(name="vpsum", bufs=4, space="PSUM"))

        # ---------------- phase 1a: load, cast, transpose, d0 projections --
        ph1 = contextlib.ExitStack()  # closed after the last filler is used
        xload = ph1.enter_context(tc.tile_pool(name="xload", bufs=2))
        wload = ph1.enter_context(tc.tile_pool(name="wload", bufs=2))
        x16_pool = ph1.enter_context(tc.tile_pool(name="x16", bufs=8))
        w16_pool = ph1.enter_context(tc.tile_pool(name="w16", bufs=4))

        # X tiles [128, 768] -> fp16
        x16 = []
        for t in range(8):
            xt_ = xload.tile([P, HIN], F32, tag="x")
            nc.sync.dma_start(xt_[:], x_ap[t * P:(t + 1) * P, :])
            x16_t = x16_pool.tile([P, HIN], F16, tag="x16", name=f"x16_{t}")
            nc.vector.tensor_copy(x16_t[:], xt_[:])
            x16.append(x16_t)

        # W loads+casts for all three weights (Wv last)
        w16 = {}
        for wname in ("q", "k", "v"):
            w_ap = aps[f"w{wname}"]
            for d in range(3):
                wt_ = wload.tile([P, HIN], F32, tag="wload")
                nc.sync.dma_start(wt_[:], w_ap[d * P:(d + 1) * P, :])
                w16_t = w16_pool.tile(
                    [P, HIN], F16, tag="w16", name=f"w16{wname}_{d}"
                )
                nc.vector.tensor_copy(w16_t[:], wt_[:])
                w16[(wname, d)] = w16_t

        # mask bias and projection biases (off the startup critical path)
        mask_i = const.tile([P, KT], I32)
        nc.sync.dma_start(mask_i[:], mask_ap.rearrange("(a p) -> p a", p=P))
        maskb = const.tile([P, KT], F32)
        nc.vector.tensor_copy(maskb[:], mask_i[:])
        nc.vector.tensor_scalar_mul(maskb[:], maskb[:], NEG)
        bias_sb = {}
        for wname in ("q", "k"):
            bt = const.tile([P, 3], F32, tag=f"b{wname}")
            nc.sync.dma_start(
                bt[:], aps[f"b{wname}"].rearrange("(a p) -> p a", p=P)
            )
            if wname == "q":
                nc.vector.tensor_scalar_mul(bt[:], bt[:], 0.125)
            bias_sb[wname] = bt
        bv_bc = const.tile([P, NH, HD], F32)
        nc.sync.dma_start(
            bv_bc[:],
            aps["bv"].rearrange("(h d) -> h d", d=HD)[None].to_broadcast(
                (P, NH, HD)
            ),
        )

        # X^T: 6 fp16 tiles [128, 1024] (h-chunk on partitions)
        xT = []
        for hc in range(6):
            pt = bigps.tile([P, S], F16, tag="ps")  # 1 bank (fp16)
            for t in range(8):
                nc.tensor.transpose(
                    pt[:, t * P:(t + 1) * P],
                    x16[t][:, hc * P:(hc + 1) * P],
                    ident[:],
                )
            xt_t = xt_pool.tile([P, S], F16, tag="xT")
            nc.scalar.copy(xt_t[:], pt[:])
            xT.append(xt_t)

        # W^T slices (fp16): wT[(w, hc)] = [128, 384]
        wT = {}
        for wname in ("q", "k", "v"):
            for hc in range(6):
                pw = bigps.tile([P, 512], F16, tag="ps", name="pw")[:, :HOUT]
                for d in range(3):
                    nc.tensor.transpose(
                        pw[:, d * P:(d + 1) * P],
                        w16[(wname, d)][:, hc * P:(hc + 1) * P],
                        ident[:],
                    )
                wt_t = wt_pool.tile([P, HOUT], F16, tag="wT")
                nc.scalar.copy(wt_t[:], pw[:])
                wT[(wname, hc)] = wt_t

        def emit_qk_proj(wname, d, tch):
            dest = qT if wname == "q" else kT
            scale = 0.125 if wname == "q" else 1.0
            pp = bigps.tile([P, 512], F32, tag="ps")
            for hc in range(6):
                nc.tensor.matmul(
                    pp[:],
                    wT[(wname, hc)][:, d * P:(d + 1) * P],
                    xT[hc][:, tch * 512:(tch + 1) * 512],
                    start=(hc == 0),
                    stop=(hc == 5),
                )
            nc.scalar.activation(
                dest[d][:, tch * 512:(tch + 1) * 512],
                pp[:],
                AF.Identity,
                bias=bias_sb[wname][:, d:d + 1],
                scale=scale,
            )

        def emit_v_proj(t):
            pv = bigps.tile([P, 512], F32, tag="ps", name="pv")[:, :HOUT]
            for hc in range(6):
                nc.tensor.matmul(
                    pv[:],
                    xT[hc][:, t * P:(t + 1) * P],
                    wT[("v", hc)][:],
                    start=(hc == 0),
                    stop=(hc == 5),
                )
            nc.vector.memset(v_tiles[t][:], 1.0)
            nc.vector.tensor_add(
                v_tiles[t][:, :, 0:HD],
                pv[:].rearrange("p (h d) -> p h d", d=HD),
                bv_bc[:],
            )

        # d=0 projections (pair 0's heads) + V tile 0 up front; the rest
        # are fillers emitted inside pair 0/1's kt blocks.
        for wname in ("q", "k"):
            for tch in range(2):
                emit_qk_proj(wname, 0, tch)
        emit_v_proj(0)

        # fillers[dt][kt] -> list of closures to emit at that block
        fillers = [[[] for _ in range(KT)] for _ in range(NPAIR)]
        for t in range(1, 8):  # V tile t needed by pair-0 block kt=t
            fillers[0][t - 1].append(lambda t=t: emit_v_proj(t))
        for i, (wname, tch) in enumerate(
            (w, t) for w in ("q", "k") for t in range(2)
        ):
            fillers[0][2 * i].append(
                lambda w=wname, t=tch: emit_qk_proj(w, 1, t)
            )
            fillers[1][2 * i].append(
                lambda w=wname, t=tch: emit_qk_proj(w, 2, t)
            )

        # transient load/cast pools are only read by the phase-1a
        # transposes; free their SBUF for the phase-2 pools
        ph1.close()

        # ---------------- phase 2: attention per head pair -----------------
        out_pool = ctx.enter_context(tc.tile_pool(name="outst", bufs=8))
        out_stage = [
            out_pool.tile([P, HOUT], F32, tag="outst", name=f"outst{i}")
            for i in range(8)
        ]
        pt_pool = ctx.enter_context(tc.tile_pool(name="pT", bufs=8))
        fin_pool = ctx.enter_context(tc.tile_pool(name="fin", bufs=4))
        ctt_pool = ctx.enter_context(tc.tile_pool(name="ctt", bufs=4))

        def emit_fin_copy(fin, ctxT_sb, h2s=(0, 1)):
            """ACT-copy the previous pair's ctx^T accumulators out of PSUM
            (releases the vpsum banks for this pair's PV groups)."""
            dt, ctxT_ps = fin
            for h2 in h2s:
                for qch in range(2):
                    t_ = ctt_pool.tile(
                        [VW, 512], F32, tag="ctxT_sb", name=f"ctT{dt}_{h2}_{qch}"
                    )
                    nc.scalar.copy(t_[:], ctxT_ps[(h2, qch)][:])
                    ctxT_sb[(h2, qch)] = t_
            return ctxT_sb

        def emit_fin_rest(fin, ctxT_sb, h2s, emit_out_dma):
            """Back-transpose ctx^T per head, divide by the denominator,
            write out_stage (and the output DMAs for the last pair)."""
            dt, _ = fin
            for h2 in h2s:
                h = 2 * dt + h2
                ctx_ps = [
                    bigps.tile([P, 512], F32, tag="ps", name=f"ctx{h}_{i}")
                    for i in range(2)
                ]
                for qt in range(QT):
                    cp = ctx_ps[qt // 4]
                    sl = (qt % 4) * VW
                    nc.tensor.transpose(
                        cp[:, sl:sl + VW],
                        ctxT_sb[(h2, qt // 4)][:, (qt % 4) * P:(qt % 4 + 1) * P],
                        ident32[:VW, :VW],
                    )
                rc4 = []
                for i in range(2):
                    rc = fin_pool.tile([P, 4], F32, tag="recip")
                    denoms = ctx_ps[i][:, 0:4 * VW].rearrange(
                        "p (a b) -> p a b", b=VW
                    )[:, :, HD]
                    nc.vector.reciprocal(rc[:], denoms)
                    rc4.append(rc)
                for qt in range(QT):
                    cp = ctx_ps[qt // 4]
                    sl = (qt % 4) * VW
                    nc.scalar.activation(
                        out_stage[qt][:, h * HD:(h + 1) * HD],
                        cp[:, sl:sl + HD],
                        AF.Identity,
                        scale=rc4[qt // 4][:, qt % 4:qt % 4 + 1],
                    )
                    if emit_out_dma and h2 == 1:
                        nc.sync.dma_start(
                            out_ap[qt * P:(qt + 1) * P, :], out_stage[qt][:]
                        )

        pending_fin = None
        for dt in range(NPAIR):
            # rel strips for both heads: [k=128, q=1024] fp32, kt-major,
            # heads interleaved to match consumption order.
            r1 = [[None] * KT for _ in range(2)]
            r2 = [[None] * KT for _ in range(2)]
            for kt in range(KT):
                for h2 in range(2):
                    h = 2 * dt + h2
                    t1 = r_pool.tile([P, S], F32, tag="rel", name=f"r1_{h}_{kt}")
                    nc.sync.dma_start(t1[:], rel1_ap[h][kt * P:(kt + 1) * P, :])
                    r1[h2][kt] = t1
                    t2 = r_pool.tile([P, S], F32, tag="rel", name=f"r2_{h}_{kt}")
                    nc.sync.dma_start(t2[:], rel2_ap[h][kt * P:(kt + 1) * P, :])
                    r2[h2][kt] = t2

            if pending_fin is not None:
                fin_sb = {}
                emit_fin_copy(pending_fin, fin_sb)

            ctxT_ps = {}
            for qch in range(2):
                for h2 in range(2):
                    ctxT_ps[(h2, qch)] = vpsum.tile(
                        [VW, 512], F32, tag="ctxT", name=f"ctxT{dt}_{h2}_{qch}"
                    )

            # kt blocks: strips fully consumed within their block; PV
            # accumulation steps interleaved so the tail after the last
            # strip is only one block's drain.
            for kt in range(KT):
                pT_kt = [
                    pt_pool.tile([P, S], F16, tag="pT", name=f"pT{dt}_{h2}_{kt}")
                    for h2 in range(2)
                ]
                for h2 in range(2):
                    if PRESUM_H2[h2]:
                        nc.gpsimd.tensor_add(
                            r1[h2][kt][:], r1[h2][kt][:], r2[h2][kt][:]
                        )
                ps4 = {}
                for qch in range(2):
                    qsl = slice(qch * 512, (qch + 1) * 512)
                    for h2 in range(2):
                        d0 = h2 * HD
                        ps = bigps.tile([P, 512], F32, tag="ps")
                        # back-to-back K=64 matmuls at base partitions 0/64
                        # get distinct row-group tile_positions -> run
                        # concurrently on the PE array
                        nc.tensor.matmul(
                            ps[:],
                            kT[dt][d0:d0 + HD, kt * P:(kt + 1) * P],
                            qT[dt][d0:d0 + HD, qsl],
                            start=True,
                            stop=True,
                        )
                        ps4[(qch, h2)] = ps
                for f in fillers[dt][kt]:
                    f()
                for qch in range(2):
                    qsl = slice(qch * 512, (qch + 1) * 512)
                    for h2 in range(2):
                        ps = ps4[(qch, h2)]
                        nc.vector.tensor_add(ps[:], ps[:], r1[h2][kt][:, qsl])
                        if not PRESUM_H2[h2]:
                            nc.vector.tensor_add(
                                ps[:], ps[:], r2[h2][kt][:, qsl]
                            )
                        nc.scalar.activation(
                            pT_kt[h2][:, qsl],
                            ps[:],
                            AF.Exp,
                            bias=maskb[:, kt:kt + 1],
                            scale=1.0,
                        )
                # PV steps for this kt (both heads x both q-chunks)
                for qch in range(2):
                    qsl = slice(qch * 512, (qch + 1) * 512)
                    for h2 in range(2):
                        h = 2 * dt + h2
                        nc.tensor.matmul(
                            ctxT_ps[(h2, qch)][:],
                            v_tiles[kt][:, h, :],
                            pT_kt[h2][:, qsl],
                            start=(kt == 0),
                            stop=(kt == KT - 1),
                            skip_group_check=True,
                        )
                # previous pair's finalize, spread mid-pair so it never
                # lands in the post-DMA tail
                if pending_fin is not None and kt in (2, 5):
                    emit_fin_rest(pending_fin, fin_sb, (kt // 3,),
                                  emit_out_dma=False)

            pending_fin = (dt, ctxT_ps)

        # last pair: per-head finalize immediately, head A first
        fin_sb = {}
        for h2 in range(2):
            emit_fin_copy(pending_fin, fin_sb, (h2,))
            emit_fin_rest(pending_fin, fin_sb, (h2,), emit_out_dma=True)


def build_program():
    """Build and compile the per-core Bass program. Returns nc."""
    nc = bacc.Bacc(
        "TRN2",
        target_bir_lowering=False,
        debug=False,
        num_devices=8,
    )
    aps = {
        "x": nc.dram_tensor("x", [S, HIN], F32, kind="ExternalInput").ap(),
        "mask": nc.dram_tensor("mask", [S], I32, kind="ExternalInput").ap(),
        "rel1": nc.dram_tensor("rel1", [NH, S, S], F32, kind="ExternalInput").ap(),
        "rel2": nc.dram_tensor("rel2", [NH, S, S], F32, kind="ExternalInput").ap(),
        "wq": nc.dram_tensor("wq", [HOUT, HIN], F32, kind="ExternalInput").ap(),
        "wk": nc.dram_tensor("wk", [HOUT, HIN], F32, kind="ExternalInput").ap(),
        "wv": nc.dram_tensor("wv", [HOUT, HIN], F32, kind="ExternalInput").ap(),
        "bq": nc.dram_tensor("bq", [HOUT], F32, kind="ExternalInput").ap(),
        "bk": nc.dram_tensor("bk", [HOUT], F32, kind="ExternalInput").ap(),
        "bv": nc.dram_tensor("bv", [HOUT], F32, kind="ExternalInput").ap(),
        "out": nc.dram_tensor("out", [S, HOUT], F32, kind="ExternalOutput").ap(),
    }
    with tile.TileContext(nc) as tc:
        _build_kernel_body(tc, aps)
    nc.compile()
    return nc


def make_in_maps(inputs):
    """Slice full inputs into the 8 per-core input maps.

    rel_pos / rel_2d_pos are uploaded transposed per head ([k, q] layout)
    so their strips add directly into the transposed score tiles."""
    hs = np.ascontiguousarray(np.asarray(inputs["hidden_states"], np.float32))
    am = np.asarray(inputs["attention_mask"]).astype(np.int32)
    rel1 = np.asarray(inputs["rel_pos"], np.float32)
    rel2 = np.asarray(inputs["rel_2d_pos"], np.float32)
    ws = {k: np.asarray(inputs["W" + k[-1]], np.float32) for k in ("wq", "wk", "wv")}
    bs = {k: np.asarray(inputs["b" + k[-1]], np.float32) for k in ("bq", "bk", "bv")}

    rel1T = np.ascontiguousarray(rel1.transpose(0, 1, 3, 2))
    rel2T = np.ascontiguousarray(rel2.transpose(0, 1, 3, 2))

    in_maps = []
    for c in range(8):
        b, hh = divmod(c, 2)
        hsl = slice(hh * NH, (hh + 1) * NH)
        csl = slice(hh * HOUT, (hh + 1) * HOUT)
        m = {
            "x": np.ascontiguousarray(hs[b]),
            "mask": np.ascontiguousarray(am[b, 0, 0]),
            "rel1": np.ascontiguousarray(rel1T[b, hsl]),
            "rel2": np.ascontiguousarray(rel2T[b, hsl]),
        }
        for k in ("wq", "wk", "wv"):
            m[k] = np.ascontiguousarray(ws[k][csl])
        for k in ("bq", "bk", "bv"):
            m[k] = np.ascontiguousarray(bs[k][csl])
        in_maps.append(m)
    return in_maps


def gather_output(results):
    out = np.empty((4, S, HIN), np.float32)
    for c in range(8):
        b, hh = divmod(c, 2)
        out[b, :, hh * HOUT:(hh + 1) * HOUT] = results[c]["out"]
    return out


_NC_CACHE = []


def kernel(**inputs):
    if not _NC_CACHE:
        _NC_CACHE.append(build_program())
    nc = _NC_CACHE[0]
    in_maps = make_in_maps(inputs)
    res = run_bass_kernel_spmd(nc, in_maps, list(range(8)))
    return gather_output(res.results)


# revision 13
# speedup vs baseline: 1.2994x; 1.0435x over previous
"""ErnieLayout self-attention on 8 Trainium2 NeuronCores (Bass/Tile). v3

Problem shapes (hardcoded): B=4, S=1024, H=768, NH=12, HD=64.
Sharding: core c -> (batch b = c//2, head-half hh = c%2, i.e. 6 heads).
Each core computes attention for its 6 heads of one batch element and
writes the [S, 384] column slice of that batch's output.

The kernel is HBM-bound: rel_pos + rel_2d_pos are 50.3 MB per core of
the ~58 MB total I/O, so the design keeps the DMA queues saturated and
sizes every engine's work under the ~160 us DMA floor (robust even when
the PE is power-throttled to 1.2 GHz, which traces show happens for most
of the kernel).

Key structure:
  * rel_pos / rel_2d_pos are uploaded HOST-TRANSPOSED per head ([k, q]
    layout, a pure layout change done while sharding).  Strips land
    contiguously; GPSIMD pre-sums rel1+rel2 in place (idle engine), and
    the DVE adds the sum straight into the transposed score PSUM with
    one RMW per [128,512] block.  No PE transposes of rel at all.
  * heads are processed in pairs (2dt, 2dt+1) whose q/k rows live in
    partitions 0-63 / 64-127 of qT/kT tile dt: the two QK score matmuls
    per (kt, qch) are emitted back-to-back and run CONCURRENTLY on the
    PE via row tiling (auto tile_position from base partitions).
  * only the X/W transposes and the d=0 Q/K projections run before the
    attention loop; the V projection and d=1,2 projections are emitted
    as fillers inside pair 0/1's kt blocks (PE slack), so pair-0
    consumption of rel strips starts ~25 us in and the 20-deep strip
    pool never backs up the DMA queue.
  * PV accumulation steps are interleaved per kt block (skip_group_
    check), so the attention tail after the last strip arrives is only
    the last block's drain + finalize.
  * scores^T layout keeps the mask as a per-partition ACT bias: masked
    keys get FLT_MIN so exp underflows to exactly 0 (no row-max needed,
    scores are O(10)).

Per-core math (identical to reference up to fp16 rounding):
  Q^T = (Wq_s @ X^T + bq)/8, K^T = Wk_s @ X^T + bk (fp16 matmuls, fp32
  PSUM), V = X @ Wv_s^T + bv stored fp16 with a ones column (col 64 ->
  softmax denominator for free).  ps[k,q] = K^T.T@Q^T (+rel12 via DVE),
  pT = exp(ps + maskbias), ctx^T[d|1, q] += V_aug[kt].T @ pT[kt],
  out[q, h*64+d] = ctx[q, d] / ctx[q, 64].
"""

import os
import sys

import numpy as np

for _p in ("/opt/trn_rl_repo",):
    if _p not in sys.path and os.path.isdir(_p):
        sys.path.append(_p)

import concourse.bass as bass
import concourse.mybir as mybir
import concourse.tile as tile
from concourse import bacc
from concourse.bass_utils import run_bass_kernel_spmd
from concourse.masks import make_identity

F32 = mybir.dt.float32
F16 = mybir.dt.float16
I32 = mybir.dt.int32
AF = mybir.ActivationFunctionType
NEG = float(np.finfo(np.float32).min)

P = 128
S = 1024
NH = 6        # heads per core
HD = 64
HIN = 768     # model dim (contraction for projections)
HOUT = NH * HD  # 384, per-core projection width
KT = S // P   # 8 key tiles
QT = S // P   # 8 query tiles
VW = HD + 1   # 65: V columns + ones column
NPAIR = NH // 2

# 'split':  GPSIMD pre-sums rel1+rel2 for head A, DVE does 2 RMWs for
#           head B (balances the two engines under the DMA pace).
# 'gpsimd': GPSIMD pre-sums everything, DVE does 1 RMW per score block.
# 'none':   DVE does 2 RMWs per score block (no pre-sum).
PRESUM = os.environ.get("K_PRESUM", "split")
PRESUM_H2 = {"gpsimd": (True, True), "split": (True, False),
             "none": (False, False)}[PRESUM]


def _build_kernel_body(tc, aps):
    import contextlib

    nc = tc.nc
    x_ap = aps["x"]
    mask_ap = aps["mask"]
    rel1_ap = aps["rel1"]  # [NH, S(k), S(q)] -- host-transposed
    rel2_ap = aps["rel2"]
    out_ap = aps["out"]

    with contextlib.ExitStack() as ctx:
        const = ctx.enter_context(tc.tile_pool(name="const", bufs=1))

        ident32 = const.tile([P, P], F32)
        make_identity(nc, ident32)

        # long-lived tensors
        qt_pool = ctx.enter_context(tc.tile_pool(name="qT", bufs=3))
        kt_pool = ctx.enter_context(tc.tile_pool(name="kT", bufs=3))
        v_pool = ctx.enter_context(tc.tile_pool(name="v", bufs=8))
        xt_pool = ctx.enter_context(tc.tile_pool(name="xT", bufs=6))
        wt_pool = ctx.enter_context(tc.tile_pool(name="wT", bufs=18))

        qT = [qt_pool.tile([P, S], F16, tag="qT", name=f"qT{i}") for i in range(3)]
        kT = [kt_pool.tile([P, S], F16, tag="kT", name=f"kT{i}") for i in range(3)]
        v_tiles = [
            v_pool.tile([P, NH, VW], F16, tag="v", name=f"v{i}") for i in range(8)
        ]

        # rel strip pool: strip DMAs queue behind the x/W loads and then
        # stream continuously for the rest of the kernel.
        r_pool = ctx.enter_context(tc.tile_pool(name="rel", bufs=26))

        # unified PSUM pools: "bigps" carries every 1-bank use (X/W
        # transpose staging, projection groups, score tiles, finalize
        # back-transposes); "vpsum" carries the 4 ctx^T accumulators.
        bigps = ctx.enter_context(tc.tile_pool(name="bigps", bufs=4, space="PSUM"))
        vpsum = ctx.enter_context(tc.tile_pool(name="vpsum", bufs=4, space="PSUM"))

        # ---------------- phase 1a: load + cast (X, W pre-transposed) ------
        ph1 = contextlib.ExitStack()  # transient fp32 landing pools
        xload = ph1.enter_context(tc.tile_pool(name="xload", bufs=2))
        wload = ph1.enter_context(tc.tile_pool(name="wload", bufs=3))

        # X^T tiles [128(hin-chunk), 1024] fp32 -> fp16 (host-transposed)
        xT = []
        for hc in range(6):
            xt_ = xload.tile([P, S], F32, tag="x")
            nc.sync.dma_start(xt_[:], x_ap[hc * P:(hc + 1) * P, :])
            xt_t = xt_pool.tile([P, S], F16, tag="xT", name=f"xT{hc}")
            nc.vector.tensor_copy(xt_t[:], xt_[:])
            xT.append(xt_t)

        # W^T tiles [128(hin-chunk), 384] fp32 -> fp16 (host-transposed)
        wT = {}
        for wname in ("q", "k", "v"):
            w_ap = aps[f"w{wname}"]
            for hc in range(6):
                wt_ = wload.tile([P, HOUT], F32, tag="wload")
                nc.sync.dma_start(wt_[:], w_ap[hc * P:(hc + 1) * P, :])
                wt_t = wt_pool.tile(
                    [P, HOUT], F16, tag="wT", name=f"wT{wname}_{hc}"
                )
                nc.vector.tensor_copy(wt_t[:], wt_[:])
                wT[(wname, hc)] = wt_t

        # mask bias and projection biases (off the startup critical path)
        mask_i = const.tile([P, KT], I32)
        nc.sync.dma_start(mask_i[:], mask_ap.rearrange("(a p) -> p a", p=P))
        maskb = const.tile([P, KT], F32)
        nc.vector.tensor_copy(maskb[:], mask_i[:])
        nc.vector.tensor_scalar_mul(maskb[:], maskb[:], NEG)
        bias_sb = {}
        for wname in ("q", "k"):
            bt = const.tile([P, 3], F32, tag=f"b{wname}")
            nc.sync.dma_start(
                bt[:], aps[f"b{wname}"].rearrange("(a p) -> p a", p=P)
            )
            if wname == "q":
                nc.vector.tensor_scalar_mul(bt[:], bt[:], 0.125)
            bias_sb[wname] = bt
        bv_bc = const.tile([P, NH, HD], F32)
        nc.sync.dma_start(
            bv_bc[:],
            aps["bv"].rearrange("(h d) -> h d", d=HD)[None].to_broadcast(
                (P, NH, HD)
            ),
        )

        def emit_qk_proj(wname, d, tch):
            dest = qT if wname == "q" else kT
            scale = 0.125 if wname == "q" else 1.0
            pp = bigps.tile([P, 512], F32, tag="ps")
            for hc in range(6):
                nc.tensor.matmul(
                    pp[:],
                    wT[(wname, hc)][:, d * P:(d + 1) * P],
                    xT[hc][:, tch * 512:(tch + 1) * 512],
                    start=(hc == 0),
                    stop=(hc == 5),
                )
            nc.scalar.activation(
                dest[d][:, tch * 512:(tch + 1) * 512],
                pp[:],
                AF.Identity,
                bias=bias_sb[wname][:, d:d + 1],
                scale=scale,
            )

        def emit_v_proj(t):
            pv = bigps.tile([P, 512], F32, tag="ps", name="pv")[:, :HOUT]
            for hc in range(6):
                nc.tensor.matmul(
                    pv[:],
                    xT[hc][:, t * P:(t + 1) * P],
                    wT[("v", hc)][:],
                    start=(hc == 0),
                    stop=(hc == 5),
                )
            nc.vector.memset(v_tiles[t][:], 1.0)
            nc.vector.tensor_add(
                v_tiles[t][:, :, 0:HD],
                pv[:].rearrange("p (h d) -> p h d", d=HD),
                bv_bc[:],
            )

        # d=0 projections (pair 0's heads) + V tile 0 up front; the rest
        # are fillers emitted inside pair 0/1's kt blocks.
        for wname in ("q", "k"):
            for tch in range(2):
                emit_qk_proj(wname, 0, tch)
        emit_v_proj(0)

        # fillers[dt][kt] -> list of closures to emit at that block
        fillers = [[[] for _ in range(KT)] for _ in range(NPAIR)]
        for t in range(1, 8):  # V tile t needed by pair-0 block kt=t
            fillers[0][t - 1].append(lambda t=t: emit_v_proj(t))
        for i, (wname, tch) in enumerate(
            (w, t) for w in ("q", "k") for t in range(2)
        ):
            fillers[0][2 * i].append(
                lambda w=wname, t=tch: emit_qk_proj(w, 1, t)
            )
            fillers[1][2 * i].append(
                lambda w=wname, t=tch: emit_qk_proj(w, 2, t)
            )

        # transient load/cast pools are only read by the phase-1a
        # transposes; free their SBUF for the phase-2 pools
        ph1.close()

        # ---------------- phase 2: attention per head pair -----------------
        out_pool = ctx.enter_context(tc.tile_pool(name="outst", bufs=8))
        out_stage = [
            out_pool.tile([P, HOUT], F32, tag="outst", name=f"outst{i}")
            for i in range(8)
        ]
        pt_pool = ctx.enter_context(tc.tile_pool(name="pT", bufs=8))
        fin_pool = ctx.enter_context(tc.tile_pool(name="fin", bufs=4))
        ctt_pool = ctx.enter_context(tc.tile_pool(name="ctt", bufs=4))

        def emit_fin_copy(fin, ctxT_sb, h2s=(0, 1)):
            """ACT-copy the previous pair's ctx^T accumulators out of PSUM
            (releases the vpsum banks for this pair's PV groups)."""
            dt, ctxT_ps = fin
            for h2 in h2s:
                for qch in range(2):
                    t_ = ctt_pool.tile(
                        [VW, 512], F32, tag="ctxT_sb", name=f"ctT{dt}_{h2}_{qch}"
                    )
                    nc.scalar.copy(t_[:], ctxT_ps[(h2, qch)][:])
                    ctxT_sb[(h2, qch)] = t_
            return ctxT_sb

        def emit_fin_rest(fin, ctxT_sb, h2s, emit_out_dma):
            """Back-transpose ctx^T per head, divide by the denominator,
            write out_stage (and the output DMAs for the last pair)."""
            dt, _ = fin
            for h2 in h2s:
                h = 2 * dt + h2
                ctx_ps = [
                    bigps.tile([P, 512], F32, tag="ps", name=f"ctx{h}_{i}")
                    for i in range(2)
                ]
                for qt in range(QT):
                    cp = ctx_ps[qt // 4]
                    sl = (qt % 4) * VW
                    nc.tensor.transpose(
                        cp[:, sl:sl + VW],
                        ctxT_sb[(h2, qt // 4)][:, (qt % 4) * P:(qt % 4 + 1) * P],
                        ident32[:VW, :VW],
                    )
                rc4 = []
                for i in range(2):
                    rc = fin_pool.tile([P, 4], F32, tag="recip")
                    denoms = ctx_ps[i][:, 0:4 * VW].rearrange(
                        "p (a b) -> p a b", b=VW
                    )[:, :, HD]
                    nc.vector.reciprocal(rc[:], denoms)
                    rc4.append(rc)
                for qt in range(QT):
                    cp = ctx_ps[qt // 4]
                    sl = (qt % 4) * VW
                    nc.scalar.activation(
                        out_stage[qt][:, h * HD:(h + 1) * HD],
                        cp[:, sl:sl + HD],
                        AF.Identity,
                        scale=rc4[qt // 4][:, qt % 4:qt % 4 + 1],
                    )
                    if emit_out_dma and h2 == 1:
                        nc.sync.dma_start(
                            out_ap[qt * P:(qt + 1) * P, :], out_stage[qt][:]
                        )

        pending_fin = None
        for dt in range(NPAIR):
            # rel strips for both heads: [k=128, q=1024] fp32, kt-major,
            # heads interleaved to match consumption order.
            r1 = [[None] * KT for _ in range(2)]
            r2 = [[None] * KT for _ in range(2)]
            for kt in range(KT):
                for h2 in range(2):
                    h = 2 * dt + h2
                    t1 = r_pool.tile([P, S], F32, tag="rel", name=f"r1_{h}_{kt}")
                    nc.sync.dma_start(t1[:], rel1_ap[h][kt * P:(kt + 1) * P, :])
                    r1[h2][kt] = t1
                    t2 = r_pool.tile([P, S], F32, tag="rel", name=f"r2_{h}_{kt}")
                    nc.sync.dma_start(t2[:], rel2_ap[h][kt * P:(kt + 1) * P, :])
                    r2[h2][kt] = t2

            if pending_fin is not None:
                fin_sb = {}
                emit_fin_copy(pending_fin, fin_sb)

            ctxT_ps = {}
            for qch in range(2):
                for h2 in range(2):
                    ctxT_ps[(h2, qch)] = vpsum.tile(
                        [VW, 512], F32, tag="ctxT", name=f"ctxT{dt}_{h2}_{qch}"
                    )

            # kt blocks: strips fully consumed within their block; PV
            # accumulation steps interleaved so the tail after the last
            # strip is only one block's drain.
            for kt in range(KT):
                pT_kt = [
                    pt_pool.tile([P, S], F16, tag="pT", name=f"pT{dt}_{h2}_{kt}")
                    for h2 in range(2)
                ]
                for h2 in range(2):
                    if PRESUM_H2[h2]:
                        nc.gpsimd.tensor_add(
                            r1[h2][kt][:], r1[h2][kt][:], r2[h2][kt][:]
                        )
                ps4 = {}
                for qch in range(2):
                    qsl = slice(qch * 512, (qch + 1) * 512)
                    for h2 in range(2):
                        d0 = h2 * HD
                        ps = bigps.tile([P, 512], F32, tag="ps")
                        # back-to-back K=64 matmuls at base partitions 0/64
                        # get distinct row-group tile_positions -> run
                        # concurrently on the PE array
                        nc.tensor.matmul(
                            ps[:],
                            kT[dt][d0:d0 + HD, kt * P:(kt + 1) * P],
                            qT[dt][d0:d0 + HD, qsl],
                            start=True,
                            stop=True,
                        )
                        ps4[(qch, h2)] = ps
                for f in fillers[dt][kt]:
                    f()
                for qch in range(2):
                    qsl = slice(qch * 512, (qch + 1) * 512)
                    for h2 in range(2):
                        ps = ps4[(qch, h2)]
                        nc.vector.tensor_add(ps[:], ps[:], r1[h2][kt][:, qsl])
                        if not PRESUM_H2[h2]:
                            nc.vector.tensor_add(
                                ps[:], ps[:], r2[h2][kt][:, qsl]
                            )
                        nc.scalar.activation(
                            pT_kt[h2][:, qsl],
                            ps[:],
                            AF.Exp,
                            bias=maskb[:, kt:kt + 1],
                            scale=1.0,
                        )
                # PV steps for this kt (both heads x both q-chunks)
                for qch in range(2):
                    qsl = slice(qch * 512, (qch + 1) * 512)
                    for h2 in range(2):
                        h = 2 * dt + h2
                        nc.tensor.matmul(
                            ctxT_ps[(h2, qch)][:],
                            v_tiles[kt][:, h, :],
                            pT_kt[h2][:, qsl],
                            start=(kt == 0),
                            stop=(kt == KT - 1),
                            skip_group_check=True,
                        )
                # previous pair's finalize, spread mid-pair so it never
                # lands in the post-DMA tail
                if pending_fin is not None and kt in (2, 5):
                    emit_fin_rest(pending_fin, fin_sb, (kt // 3,),
                                  emit_out_dma=False)

            pending_fin = (dt, ctxT_ps)

        # last pair: per-head finalize immediately, head A first
        fin_sb = {}
        for h2 in range(2):
            emit_fin_copy(pending_fin, fin_sb, (h2,))
            emit_fin_rest(pending_fin, fin_sb, (h2,), emit_out_dma=True)


def build_program():
    """Build and compile the per-core Bass program. Returns nc."""
    nc = bacc.Bacc(
        "TRN2",
        target_bir_lowering=False,
        debug=False,
        num_devices=8,
    )
    aps = {
        "x": nc.dram_tensor("x", [HIN, S], F32, kind="ExternalInput").ap(),
        "mask": nc.dram_tensor("mask", [S], I32, kind="ExternalInput").ap(),
        "rel1": nc.dram_tensor("rel1", [NH, S, S], F32, kind="ExternalInput").ap(),
        "rel2": nc.dram_tensor("rel2", [NH, S, S], F32, kind="ExternalInput").ap(),
        "wq": nc.dram_tensor("wq", [HIN, HOUT], F32, kind="ExternalInput").ap(),
        "wk": nc.dram_tensor("wk", [HIN, HOUT], F32, kind="ExternalInput").ap(),
        "wv": nc.dram_tensor("wv", [HIN, HOUT], F32, kind="ExternalInput").ap(),
        "bq": nc.dram_tensor("bq", [HOUT], F32, kind="ExternalInput").ap(),
        "bk": nc.dram_tensor("bk", [HOUT], F32, kind="ExternalInput").ap(),
        "bv": nc.dram_tensor("bv", [HOUT], F32, kind="ExternalInput").ap(),
        "out": nc.dram_tensor("out", [S, HOUT], F32, kind="ExternalOutput").ap(),
    }
    with tile.TileContext(nc) as tc:
        _build_kernel_body(tc, aps)
    nc.compile()
    return nc


def make_in_maps(inputs):
    """Slice full inputs into the 8 per-core input maps.

    rel_pos / rel_2d_pos are uploaded transposed per head ([k, q] layout)
    so their strips add directly into the transposed score tiles."""
    hs = np.ascontiguousarray(np.asarray(inputs["hidden_states"], np.float32))
    am = np.asarray(inputs["attention_mask"]).astype(np.int32)
    rel1 = np.asarray(inputs["rel_pos"], np.float32)
    rel2 = np.asarray(inputs["rel_2d_pos"], np.float32)
    ws = {k: np.asarray(inputs["W" + k[-1]], np.float32) for k in ("wq", "wk", "wv")}
    bs = {k: np.asarray(inputs["b" + k[-1]], np.float32) for k in ("bq", "bk", "bv")}

    rel1T = np.ascontiguousarray(rel1.transpose(0, 1, 3, 2))
    rel2T = np.ascontiguousarray(rel2.transpose(0, 1, 3, 2))

    in_maps = []
    for c in range(8):
        b, hh = divmod(c, 2)
        hsl = slice(hh * NH, (hh + 1) * NH)
        csl = slice(hh * HOUT, (hh + 1) * HOUT)
        m = {
            "x": np.ascontiguousarray(hs[b].T),
            "mask": np.ascontiguousarray(am[b, 0, 0]),
            "rel1": np.ascontiguousarray(rel1T[b, hsl]),
            "rel2": np.ascontiguousarray(rel2T[b, hsl]),
        }
        for k in ("wq", "wk", "wv"):
            m[k] = np.ascontiguousarray(ws[k][csl].T)
        for k in ("bq", "bk", "bv"):
            m[k] = np.ascontiguousarray(bs[k][csl])
        in_maps.append(m)
    return in_maps


def gather_output(results):
    out = np.empty((4, S, HIN), np.float32)
    for c in range(8):
        b, hh = divmod(c, 2)
        out[b, :, hh * HOUT:(hh + 1) * HOUT] = results[c]["out"]
    return out


_NC_CACHE = []


def kernel(**inputs):
    if not _NC_CACHE:
        _NC_CACHE.append(build_program())
    nc = _NC_CACHE[0]
    in_maps = make_in_maps(inputs)
    res = run_bass_kernel_spmd(nc, in_maps, list(range(8)))
    return gather_output(res.results)


# revision 15
# speedup vs baseline: 1.3725x; 1.0563x over previous
"""ErnieLayout self-attention on 8 Trainium2 NeuronCores (Bass/Tile). v3

Problem shapes (hardcoded): B=4, S=1024, H=768, NH=12, HD=64.
Sharding: core c -> (batch b = c//2, head-half hh = c%2, i.e. 6 heads).
Each core computes attention for its 6 heads of one batch element and
writes the [S, 384] column slice of that batch's output.

The kernel is HBM-bound: rel_pos + rel_2d_pos are 50.3 MB per core of
the ~58 MB total I/O, so the design keeps the DMA queues saturated and
sizes every engine's work under the ~160 us DMA floor (robust even when
the PE is power-throttled to 1.2 GHz, which traces show happens for most
of the kernel).

Key structure:
  * rel_pos / rel_2d_pos are uploaded HOST-TRANSPOSED per head ([k, q]
    layout, a pure layout change done while sharding).  Strips land
    contiguously; GPSIMD pre-sums rel1+rel2 in place (idle engine), and
    the DVE adds the sum straight into the transposed score PSUM with
    one RMW per [128,512] block.  No PE transposes of rel at all.
  * heads are processed in pairs (2dt, 2dt+1) whose q/k rows live in
    partitions 0-63 / 64-127 of qT/kT tile dt: the two QK score matmuls
    per (kt, qch) are emitted back-to-back and run CONCURRENTLY on the
    PE via row tiling (auto tile_position from base partitions).
  * only the X/W transposes and the d=0 Q/K projections run before the
    attention loop; the V projection and d=1,2 projections are emitted
    as fillers inside pair 0/1's kt blocks (PE slack), so pair-0
    consumption of rel strips starts ~25 us in and the 20-deep strip
    pool never backs up the DMA queue.
  * PV accumulation steps are interleaved per kt block (skip_group_
    check), so the attention tail after the last strip arrives is only
    the last block's drain + finalize.
  * scores^T layout keeps the mask as a per-partition ACT bias: masked
    keys get FLT_MIN so exp underflows to exactly 0 (no row-max needed,
    scores are O(10)).

Per-core math (identical to reference up to fp16 rounding):
  Q^T = (Wq_s @ X^T + bq)/8, K^T = Wk_s @ X^T + bk (fp16 matmuls, fp32
  PSUM), V = X @ Wv_s^T + bv stored fp16 with a ones column (col 64 ->
  softmax denominator for free).  ps[k,q] = K^T.T@Q^T (+rel12 via DVE),
  pT = exp(ps + maskbias), ctx^T[d|1, q] += V_aug[kt].T @ pT[kt],
  out[q, h*64+d] = ctx[q, d] / ctx[q, 64].
"""

import os
import sys

import numpy as np

for _p in ("/opt/trn_rl_repo",):
    if _p not in sys.path and os.path.isdir(_p):
        sys.path.append(_p)

import concourse.bass as bass
import concourse.mybir as mybir
import concourse.tile as tile
from concourse import bacc
from concourse.bass_utils import run_bass_kernel_spmd
from concourse.masks import make_identity

F32 = mybir.dt.float32
F16 = mybir.dt.float16
I32 = mybir.dt.int32
AF = mybir.ActivationFunctionType
NEG = float(np.finfo(np.float32).min)

P = 128
S = 1024
NH = 6        # heads per core
HD = 64
HIN = 768     # model dim (contraction for projections)
HOUT = NH * HD  # 384, per-core projection width
KT = S // P   # 8 key tiles
QT = S // P   # 8 query tiles
VW = HD + 1   # 65: V columns + ones column
NPAIR = NH // 2

# 'split':  GPSIMD pre-sums rel1+rel2 for head A, DVE does 2 RMWs for
#           head B (balances the two engines under the DMA pace).
# 'gpsimd': GPSIMD pre-sums everything, DVE does 1 RMW per score block.
# 'none':   DVE does 2 RMWs per score block (no pre-sum).
PRESUM = os.environ.get("K_PRESUM", "split")
PRESUM_H2 = {"gpsimd": (True, True), "split": (True, False),
             "none": (False, False)}[PRESUM]


def _build_kernel_body(tc, aps):
    import contextlib

    nc = tc.nc
    x_ap = aps["x"]
    mask_ap = aps["mask"]
    rel1_ap = aps["rel1"]  # [NH, S(k), S(q)] -- host-transposed
    rel2_ap = aps["rel2"]
    out_ap = aps["out"]

    with contextlib.ExitStack() as ctx:
        const = ctx.enter_context(tc.tile_pool(name="const", bufs=1))

        ident32 = const.tile([P, P], F32)
        make_identity(nc, ident32)

        # long-lived tensors
        qt_pool = ctx.enter_context(tc.tile_pool(name="qT", bufs=3))
        kt_pool = ctx.enter_context(tc.tile_pool(name="kT", bufs=3))
        v_pool = ctx.enter_context(tc.tile_pool(name="v", bufs=8))
        xt_pool = ctx.enter_context(tc.tile_pool(name="xT", bufs=6))
        wt_pool = ctx.enter_context(tc.tile_pool(name="wT", bufs=6))

        qT = [qt_pool.tile([P, S], F16, tag="qT", name=f"qT{i}") for i in range(3)]
        kT = [kt_pool.tile([P, S], F16, tag="kT", name=f"kT{i}") for i in range(3)]
        v_tiles = [
            v_pool.tile([P, NH, VW], F16, tag="v", name=f"v{i}") for i in range(8)
        ]

        # rel strip pool: strip DMAs queue behind the x/W loads and then
        # stream continuously for the rest of the kernel.
        r_pool = ctx.enter_context(tc.tile_pool(name="rel", bufs=26))

        # unified PSUM pools: "bigps" carries every 1-bank use (X/W
        # transpose staging, projection groups, score tiles, finalize
        # back-transposes); "vpsum" carries the 4 ctx^T accumulators.
        bigps = ctx.enter_context(tc.tile_pool(name="bigps", bufs=4, space="PSUM"))
        vpsum = ctx.enter_context(tc.tile_pool(name="vpsum", bufs=4, space="PSUM"))

        # ---------------- phase 1a: load + cast (X, W pre-transposed) ------
        ph1 = contextlib.ExitStack()  # transient fp32 landing pools
        xload = ph1.enter_context(tc.tile_pool(name="xload", bufs=2))
        wload = ph1.enter_context(tc.tile_pool(name="wload", bufs=2))

        # X^T tiles [128(hin-chunk), 1024] fp32 -> fp16 (host-transposed)
        xT = []
        for hc in range(6):
            xt_ = xload.tile([P, S], F32, tag="x")
            nc.sync.dma_start(xt_[:], x_ap[hc * P:(hc + 1) * P, :])
            xt_t = xt_pool.tile([P, S], F16, tag="xT", name=f"xT{hc}")
            nc.vector.tensor_copy(xt_t[:], xt_[:])
            xT.append(xt_t)

        # W^T tiles: host packs [wq^T | wk^T | wv^T] row-wise into one
        # [HIN, 3*384] tensor -> 6 big loads [128, 1152] fp32 -> fp16
        wqkv_ap = aps["wqkv"]
        wT = {}
        for hc in range(6):
            wt_ = wload.tile([P, 3, HOUT], F32, tag="wload")
            nc.sync.dma_start(wt_[:], wqkv_ap[hc * P:(hc + 1) * P, :, :])
            wt_t = wt_pool.tile(
                [P, 3, HOUT], F16, tag="wT", name=f"wT{hc}"
            )
            nc.vector.tensor_copy(wt_t[:], wt_[:])
            for wi, wname in enumerate(("q", "k", "v")):
                wT[(wname, hc)] = wt_t[:, wi, :]

        # mask bias and projection biases (off the startup critical path)
        mask_i = const.tile([P, KT], I32)
        nc.sync.dma_start(mask_i[:], mask_ap.rearrange("(a p) -> p a", p=P))
        maskb = const.tile([P, KT], F32)
        nc.vector.tensor_copy(maskb[:], mask_i[:])
        nc.vector.tensor_scalar_mul(maskb[:], maskb[:], NEG)
        bias_sb = {}
        for wname in ("q", "k"):
            bt = const.tile([P, 3], F32, tag=f"b{wname}")
            nc.sync.dma_start(
                bt[:], aps[f"b{wname}"].rearrange("(a p) -> p a", p=P)
            )
            if wname == "q":
                nc.vector.tensor_scalar_mul(bt[:], bt[:], 0.125)
            bias_sb[wname] = bt
        bv_bc = const.tile([P, NH, HD], F32)
        nc.sync.dma_start(
            bv_bc[:],
            aps["bv"].rearrange("(h d) -> h d", d=HD)[None].to_broadcast(
                (P, NH, HD)
            ),
        )

        def emit_qk_proj(wname, d, tch):
            dest = qT if wname == "q" else kT
            scale = 0.125 if wname == "q" else 1.0
            pp = bigps.tile([P, 512], F32, tag="ps")
            for hc in range(6):
                nc.tensor.matmul(
                    pp[:],
                    wT[(wname, hc)][:, d * P:(d + 1) * P],
                    xT[hc][:, tch * 512:(tch + 1) * 512],
                    start=(hc == 0),
                    stop=(hc == 5),
                )
            nc.scalar.activation(
                dest[d][:, tch * 512:(tch + 1) * 512],
                pp[:],
                AF.Identity,
                bias=bias_sb[wname][:, d:d + 1],
                scale=scale,
            )

        def emit_v_proj(t):
            pv = bigps.tile([P, 512], F32, tag="ps", name="pv")[:, :HOUT]
            for hc in range(6):
                nc.tensor.matmul(
                    pv[:],
                    xT[hc][:, t * P:(t + 1) * P],
                    wT[("v", hc)][:],
                    start=(hc == 0),
                    stop=(hc == 5),
                )
            nc.vector.memset(v_tiles[t][:], 1.0)
            nc.vector.tensor_add(
                v_tiles[t][:, :, 0:HD],
                pv[:].rearrange("p (h d) -> p h d", d=HD),
                bv_bc[:],
            )

        # d=0 projections (pair 0's heads) + V tile 0 up front; the rest
        # are fillers emitted inside pair 0/1's kt blocks.
        for wname in ("q", "k"):
            for tch in range(2):
                emit_qk_proj(wname, 0, tch)
        emit_v_proj(0)

        # fillers[dt][kt] -> list of closures to emit at that block
        fillers = [[[] for _ in range(KT)] for _ in range(NPAIR)]
        for t in range(1, 8):  # V tile t needed by pair-0 block kt=t
            fillers[0][t - 1].append(lambda t=t: emit_v_proj(t))
        for i, (wname, tch) in enumerate(
            (w, t) for w in ("q", "k") for t in range(2)
        ):
            fillers[0][2 * i].append(
                lambda w=wname, t=tch: emit_qk_proj(w, 1, t)
            )
            fillers[1][2 * i].append(
                lambda w=wname, t=tch: emit_qk_proj(w, 2, t)
            )

        # transient load/cast pools are only read by the phase-1a
        # transposes; free their SBUF for the phase-2 pools
        ph1.close()

        # ---------------- phase 2: attention per head pair -----------------
        out_pool = ctx.enter_context(tc.tile_pool(name="outst", bufs=8))
        out_stage = [
            out_pool.tile([P, HOUT], F32, tag="outst", name=f"outst{i}")
            for i in range(8)
        ]
        pt_pool = ctx.enter_context(tc.tile_pool(name="pT", bufs=8))
        fin_pool = ctx.enter_context(tc.tile_pool(name="fin", bufs=4))
        ctt_pool = ctx.enter_context(tc.tile_pool(name="ctt", bufs=4))

        def emit_fin_copy(fin, ctxT_sb, h2s=(0, 1)):
            """ACT-copy the previous pair's ctx^T accumulators out of PSUM
            (releases the vpsum banks for this pair's PV groups)."""
            dt, ctxT_ps = fin
            for h2 in h2s:
                for qch in range(2):
                    t_ = ctt_pool.tile(
                        [VW, 512], F32, tag="ctxT_sb", name=f"ctT{dt}_{h2}_{qch}"
                    )
                    nc.scalar.copy(t_[:], ctxT_ps[(h2, qch)][:])
                    ctxT_sb[(h2, qch)] = t_
            return ctxT_sb

        def emit_fin_rest(fin, ctxT_sb, h2s, emit_out_dma):
            """Back-transpose ctx^T per head, divide by the denominator,
            write out_stage (and the output DMAs for the last pair)."""
            dt, _ = fin
            for h2 in h2s:
                h = 2 * dt + h2
                ctx_ps = [
                    bigps.tile([P, 512], F32, tag="ps", name=f"ctx{h}_{i}")
                    for i in range(2)
                ]
                for qt in range(QT):
                    cp = ctx_ps[qt // 4]
                    sl = (qt % 4) * VW
                    nc.tensor.transpose(
                        cp[:, sl:sl + VW],
                        ctxT_sb[(h2, qt // 4)][:, (qt % 4) * P:(qt % 4 + 1) * P],
                        ident32[:VW, :VW],
                    )
                rc4 = []
                for i in range(2):
                    rc = fin_pool.tile([P, 4], F32, tag="recip")
                    denoms = ctx_ps[i][:, 0:4 * VW].rearrange(
                        "p (a b) -> p a b", b=VW
                    )[:, :, HD]
                    nc.vector.reciprocal(rc[:], denoms)
                    rc4.append(rc)
                for qt in range(QT):
                    cp = ctx_ps[qt // 4]
                    sl = (qt % 4) * VW
                    nc.scalar.activation(
                        out_stage[qt][:, h * HD:(h + 1) * HD],
                        cp[:, sl:sl + HD],
                        AF.Identity,
                        scale=rc4[qt // 4][:, qt % 4:qt % 4 + 1],
                    )
                    if emit_out_dma and h2 == 1:
                        nc.sync.dma_start(
                            out_ap[qt * P:(qt + 1) * P, :], out_stage[qt][:]
                        )

        pending_fin = None
        for dt in range(NPAIR):
            # rel strips for both heads: [k=128, q=1024] fp32, kt-major,
            # heads interleaved to match consumption order.
            r1 = [[None] * KT for _ in range(2)]
            r2 = [[None] * KT for _ in range(2)]
            for kt in range(KT):
                for h2 in range(2):
                    h = 2 * dt + h2
                    t1 = r_pool.tile([P, S], F32, tag="rel", name=f"r1_{h}_{kt}")
                    nc.sync.dma_start(t1[:], rel1_ap[h][kt * P:(kt + 1) * P, :])
                    r1[h2][kt] = t1
                    t2 = r_pool.tile([P, S], F32, tag="rel", name=f"r2_{h}_{kt}")
                    nc.sync.dma_start(t2[:], rel2_ap[h][kt * P:(kt + 1) * P, :])
                    r2[h2][kt] = t2

            if pending_fin is not None:
                fin_sb = {}
                emit_fin_copy(pending_fin, fin_sb)

            ctxT_ps = {}
            for qch in range(2):
                for h2 in range(2):
                    ctxT_ps[(h2, qch)] = vpsum.tile(
                        [VW, 512], F32, tag="ctxT", name=f"ctxT{dt}_{h2}_{qch}"
                    )

            # kt blocks: strips fully consumed within their block; PV
            # accumulation steps interleaved so the tail after the last
            # strip is only one block's drain.
            for kt in range(KT):
                pT_kt = [
                    pt_pool.tile([P, S], F16, tag="pT", name=f"pT{dt}_{h2}_{kt}")
                    for h2 in range(2)
                ]
                for h2 in range(2):
                    if PRESUM_H2[h2]:
                        nc.gpsimd.tensor_add(
                            r1[h2][kt][:], r1[h2][kt][:], r2[h2][kt][:]
                        )
                ps4 = {}
                for qch in range(2):
                    qsl = slice(qch * 512, (qch + 1) * 512)
                    for h2 in range(2):
                        d0 = h2 * HD
                        ps = bigps.tile([P, 512], F32, tag="ps")
                        # back-to-back K=64 matmuls at base partitions 0/64
                        # get distinct row-group tile_positions -> run
                        # concurrently on the PE array
                        nc.tensor.matmul(
                            ps[:],
                            kT[dt][d0:d0 + HD, kt * P:(kt + 1) * P],
                            qT[dt][d0:d0 + HD, qsl],
                            start=True,
                            stop=True,
                        )
                        ps4[(qch, h2)] = ps
                for f in fillers[dt][kt]:
                    f()
                for qch in range(2):
                    qsl = slice(qch * 512, (qch + 1) * 512)
                    for h2 in range(2):
                        ps = ps4[(qch, h2)]
                        nc.vector.tensor_add(ps[:], ps[:], r1[h2][kt][:, qsl])
                        if not PRESUM_H2[h2]:
                            nc.vector.tensor_add(
                                ps[:], ps[:], r2[h2][kt][:, qsl]
                            )
                        nc.scalar.activation(
                            pT_kt[h2][:, qsl],
                            ps[:],
                            AF.Exp,
                            bias=maskb[:, kt:kt + 1],
                            scale=1.0,
                        )
                # PV steps for this kt (both heads x both q-chunks)
                for qch in range(2):
                    qsl = slice(qch * 512, (qch + 1) * 512)
                    for h2 in range(2):
                        h = 2 * dt + h2
                        nc.tensor.matmul(
                            ctxT_ps[(h2, qch)][:],
                            v_tiles[kt][:, h, :],
                            pT_kt[h2][:, qsl],
                            start=(kt == 0),
                            stop=(kt == KT - 1),
                            skip_group_check=True,
                        )
                # previous pair's finalize, spread mid-pair so it never
                # lands in the post-DMA tail
                if pending_fin is not None and kt in (2, 5):
                    emit_fin_rest(pending_fin, fin_sb, (kt // 3,),
                                  emit_out_dma=False)

            pending_fin = (dt, ctxT_ps)

        # last pair: copies on the DVE (ACT is draining exps), then both
        # heads' scales interleaved per q-tile with its output DMA right
        # behind, so the store stream pipelines with the scale stream.
        dt_l, ctxT_l = pending_fin
        fin_sb = {}
        for h2 in range(2):
            for qch in range(2):
                t_ = ctt_pool.tile(
                    [VW, 512], F32, tag="ctxT_sb", name=f"ctTL_{h2}_{qch}"
                )
                nc.vector.tensor_copy(t_[:], ctxT_l[(h2, qch)][:])
                fin_sb[(h2, qch)] = t_
        ctx_ps_l = {}
        rc4_l = {}
        for h2 in range(2):
            cps = [
                bigps.tile([P, 512], F32, tag="ps", name=f"lctx{h2}_{i}")
                for i in range(2)
            ]
            for qt in range(QT):
                cp = cps[qt // 4]
                sl = (qt % 4) * VW
                nc.tensor.transpose(
                    cp[:, sl:sl + VW],
                    fin_sb[(h2, qt // 4)][:, (qt % 4) * P:(qt % 4 + 1) * P],
                    ident32[:VW, :VW],
                )
            for i in range(2):
                rc = fin_pool.tile([P, 4], F32, tag="recip")
                denoms = cps[i][:, 0:4 * VW].rearrange(
                    "p (a b) -> p a b", b=VW
                )[:, :, HD]
                nc.vector.reciprocal(rc[:], denoms)
                rc4_l[(h2, i)] = rc
            ctx_ps_l[h2] = cps
        for qt in range(QT):
            for h2 in range(2):
                h = 2 * dt_l + h2
                cp = ctx_ps_l[h2][qt // 4]
                sl = (qt % 4) * VW
                nc.scalar.activation(
                    out_stage[qt][:, h * HD:(h + 1) * HD],
                    cp[:, sl:sl + HD],
                    AF.Identity,
                    scale=rc4_l[(h2, qt // 4)][:, qt % 4:qt % 4 + 1],
                )
            nc.sync.dma_start(
                out_ap[qt * P:(qt + 1) * P, :], out_stage[qt][:]
            )


def build_program():
    """Build and compile the per-core Bass program. Returns nc."""
    nc = bacc.Bacc(
        "TRN2",
        target_bir_lowering=False,
        debug=False,
        num_devices=8,
    )
    aps = {
        "x": nc.dram_tensor("x", [HIN, S], F32, kind="ExternalInput").ap(),
        "mask": nc.dram_tensor("mask", [S], I32, kind="ExternalInput").ap(),
        "rel1": nc.dram_tensor("rel1", [NH, S, S], F32, kind="ExternalInput").ap(),
        "rel2": nc.dram_tensor("rel2", [NH, S, S], F32, kind="ExternalInput").ap(),
        "wqkv": nc.dram_tensor(
            "wqkv", [HIN, 3, HOUT], F32, kind="ExternalInput"
        ).ap(),
        "bq": nc.dram_tensor("bq", [HOUT], F32, kind="ExternalInput").ap(),
        "bk": nc.dram_tensor("bk", [HOUT], F32, kind="ExternalInput").ap(),
        "bv": nc.dram_tensor("bv", [HOUT], F32, kind="ExternalInput").ap(),
        "out": nc.dram_tensor("out", [S, HOUT], F32, kind="ExternalOutput").ap(),
    }
    with tile.TileContext(nc) as tc:
        _build_kernel_body(tc, aps)
    nc.compile()
    return nc


def make_in_maps(inputs):
    """Slice full inputs into the 8 per-core input maps.

    rel_pos / rel_2d_pos are uploaded transposed per head ([k, q] layout)
    so their strips add directly into the transposed score tiles."""
    hs = np.ascontiguousarray(np.asarray(inputs["hidden_states"], np.float32))
    am = np.asarray(inputs["attention_mask"]).astype(np.int32)
    rel1 = np.asarray(inputs["rel_pos"], np.float32)
    rel2 = np.asarray(inputs["rel_2d_pos"], np.float32)
    ws = {k: np.asarray(inputs["W" + k[-1]], np.float32) for k in ("wq", "wk", "wv")}
    bs = {k: np.asarray(inputs["b" + k[-1]], np.float32) for k in ("bq", "bk", "bv")}

    rel1T = np.ascontiguousarray(rel1.transpose(0, 1, 3, 2))
    rel2T = np.ascontiguousarray(rel2.transpose(0, 1, 3, 2))

    in_maps = []
    for c in range(8):
        b, hh = divmod(c, 2)
        hsl = slice(hh * NH, (hh + 1) * NH)
        csl = slice(hh * HOUT, (hh + 1) * HOUT)
        m = {
            "x": np.ascontiguousarray(hs[b].T),
            "mask": np.ascontiguousarray(am[b, 0, 0]),
            "rel1": np.ascontiguousarray(rel1T[b, hsl]),
            "rel2": np.ascontiguousarray(rel2T[b, hsl]),
        }
        m["wqkv"] = np.ascontiguousarray(
            np.stack([ws[k][csl].T for k in ("wq", "wk", "wv")], axis=1)
        )
        for k in ("bq", "bk", "bv"):
            m[k] = np.ascontiguousarray(bs[k][csl])
        in_maps.append(m)
    return in_maps


def gather_output(results):
    out = np.empty((4, S, HIN), np.float32)
    for c in range(8):
        b, hh = divmod(c, 2)
        out[b, :, hh * HOUT:(hh + 1) * HOUT] = results[c]["out"]
    return out


_NC_CACHE = []


def kernel(**inputs):
    if not _NC_CACHE:
        _NC_CACHE.append(build_program())
    nc = _NC_CACHE[0]
    in_maps = make_in_maps(inputs)
    res = run_bass_kernel_spmd(nc, in_maps, list(range(8)))
    return gather_output(res.results)
